# revision 1
# baseline (speedup 1.0000x reference)
"""Trainium2 Bass kernel for attribute visual attention.

Computes, for each batch b:
    q      = v @ W_alpha                  # [i, f]
    scores = q @ vf[b]                    # [i, r]
    atten  = softmax(scores, axis=r)
    out[b] = atten @ vf[b].T              # [i, f]

Sharding: data-parallel over batch b across 8 NeuronCores (8 batches per
core); v / W_alpha replicated. All matmuls run in fp16 (full PE rate on
TRN2) with fp32 PSUM accumulation; softmax statistics in fp32.

Layout notes:
- The attend matmul contracts over r, which must live on SBUF partitions
  for both operands; the host passes visual_features twice — [f, r] for
  the scores matmul and pre-transposed [r, f] for the attend matmul. The
  small e = exp(scores - max) matrix is transposed on-chip with the DMA
  xbar (fp16).
- Batches are processed in PAIRS for the scores matmul (rhs = two
  batches side by side, N=392): halves the number of PE instructions and
  stationary-weight loads.
- Bulk HBM traffic uses SWDGE (gpsimd) so the shared HWDGE block is left
  for the xbar transposes.
- Softmax normalization is folded into the PSUM->SBUF output copy as a
  per-partition scale.
"""

import numpy as np
from contextlib import ExitStack

import concourse.bass as bass
import concourse.tile as tile
import concourse.bass_utils as bass_utils
from concourse import bacc, mybir

# Problem shapes (hardcoded per contest contract).
B, F, R, I, V = 64, 2048, 196, 312, 300
NCORES = 8
BL = B // NCORES          # 8 batches per core
NPAIR = BL // 2           # 4 batch-pairs per core
FT = F // 128             # 16 f-tiles
RPAD = 256                # r padded to 2x128 for the xbar transpose
I_TILES = ((0, 128), (128, 128), (256, 56))
KV_TILES = ((0, 128), (128, 128), (256, 44))    # v=300
KR_TILES = ((0, 128), (128, 68))                # r=196

F16 = mybir.dt.float16
F32 = mybir.dt.float32

_CACHE = {}


def _build_body(nc, tc, ctx, wa, vt, vf, vft, ident, out, reps):
    qtp = ctx.enter_context(tc.tile_pool(name="qt", bufs=1))
    ident_t = qtp.tile([128, 128], F16, tag="ident", name="ident")
    with tc.high_priority():
        nc.sync.dma_start(ident_t[:], ident[:])

    # PE warm-up: ~30 junk matmuls on the identity while the weight loads are
    # still in flight, so the clock ramp completes before real work starts
    with tc.tile_pool(name="wupsum", bufs=1, space=bass.MemorySpace.PSUM) as wup:
        wu = wup.tile([128, 128], F32, tag="wu", name="wu")
        for w in range(55):
            nc.tensor.matmul(wu[:], ident_t[:], ident_t[:],
                             start=(w == 0), stop=(w == 54))

    # ---- Phase 0: qT[f, i] = (v @ W_alpha).T via lhsT=W_alpha, rhs=v.T ----
    qt_t = []
    with tc.tile_pool(name="const", bufs=1) as const, \
         tc.tile_pool(name="qpsum", bufs=2, space=bass.MemorySpace.PSUM) as qpsum:
        wa_t, vt_t = [], []
        for k, (v0, vs) in enumerate(KV_TILES):
            t = const.tile([vs, I], F16, tag=f"vt{k}")
            with tc.high_priority():
                nc.sync.dma_start(t[:], vt[v0:v0 + vs, :])
            vt_t.append(t)
        for k, (v0, vs) in enumerate(KV_TILES):
            w = const.tile([vs, F], F16, tag=f"wa{k}")
            with tc.high_priority():
                for c in range(2):
                    nc.sync.dma_start(w[:, c * 1024:(c + 1) * 1024],
                                      wa[v0:v0 + vs, c * 1024:(c + 1) * 1024])
            wa_t.append(w)

        for mf in range(FT):
            qp = qpsum.tile([128, I], F32, tag="qp")
            for k, (v0, vs) in enumerate(KV_TILES):
                nc.tensor.matmul(qp[:], wa_t[k][:, mf * 128:(mf + 1) * 128],
                                 vt_t[k][:], start=(k == 0), stop=(k == 2))
            q = qtp.tile([128, I], F16, tag=f"qt{mf}")
            nc.scalar.copy(q[:], qp[:])
            qt_t.append(q)

    # ---- Phase 1: per batch-pair attention ----
    vfp = ctx.enter_context(tc.tile_pool(name="vf", bufs=4))
    vftp = ctx.enter_context(tc.tile_pool(name="vft", bufs=4))
    esp = ctx.enter_context(tc.tile_pool(name="es", bufs=6))
    attp = ctx.enter_context(tc.tile_pool(name="atT", bufs=3))
    outp = ctx.enter_context(tc.tile_pool(name="out", bufs=2))
    stat = ctx.enter_context(tc.tile_pool(name="stat", bufs=8))
    spsum = ctx.enter_context(
        tc.tile_pool(name="spsum", bufs=3, space=bass.MemorySpace.PSUM))
    opsum = ctx.enter_context(
        tc.tile_pool(name="opsum", bufs=4, space=bass.MemorySpace.PSUM))
    tpsum = ctx.enter_context(
        tc.tile_pool(name="tpsum", bufs=1, space=bass.MemorySpace.PSUM))

    PW = 1     # pairs per wave
    for rep in range(reps):
        for half in range(NPAIR // PW):
            if half > 0:
                # PSUM-free PE activity across the DMA-bound wave boundary:
                # standalone weight loads keep the clock-ramp monitor fed
                for _ in range(10):
                    nc.tensor.ldweights(ident_t[:])
            # vf pair tiles: [128, t, j*196+r] for the wave's batch pairs
            vf_t, vft_t = [], {}
            for p in range(PW):
                bp = half * PW + p
                vt_ = vfp.tile([128, FT, 2 * R], F16, tag="vf", name=f"vf{p}")
                with tc.high_priority():
                    for c in range(4):
                        nc.gpsimd.dma_start(vt_[:, 4 * c:4 * (c + 1), :],
                                            vf[bp, :, 4 * c:4 * (c + 1), :])
                vf_t.append(vt_)
                for j in range(2):
                    b = 2 * bp + j
                    jj = 2 * p + j
                    for kr, (r0, rs) in enumerate(KR_TILES):
                        vv = vftp.tile([rs, F], F16, tag=f"vft{kr}{jj}",
                                       name=f"vft{kr}{jj}")
                        with tc.high_priority():
                            nc.gpsimd.dma_start(vv[:], vft[b, r0:r0 + rs, :])
                        vft_t[(jj, kr)] = vv

            esT_full = [
                [attp.tile([rs, I], F16, tag=f"esT{kr}{jj % 2}",
                           name=f"esT{kr}{jj % 2}")
                 for kr, (r0, rs) in enumerate(KR_TILES)]
                for jj in range(2 * PW)]
            for mi, (i0, isz) in enumerate(I_TILES):
                # scores for all wave batches; inner loop over pairs so the
                # stationary qT tile is reused PW times per load
                sps = [spsum.tile([isz, 2, R], F32, tag="sp", name=f"sp{p}")
                       for p in range(PW)]
                for kf in range(FT):
                    for p in range(PW):
                        nc.tensor.matmul(
                            sps[p][:], qt_t[kf][:, i0:i0 + isz],
                            vf_t[p][:, kf, :].rearrange("p (j r) -> p j r", j=2),
                            start=(kf == 0), stop=(kf == FT - 1))

                for p in range(PW):
                    sp = sps[p]
                    negmax = stat.tile([isz, 2], F32, tag="negmax")
                    with tc.high_priority():
                        nc.vector.tensor_reduce(negmax[:], sp[:],
                                                axis=mybir.AxisListType.X,
                                                op=mybir.AluOpType.max, negate=True)
                    sums = stat.tile([isz, 2], F32, tag="sums")
                    rcp = stat.tile([isz, 2], F32, tag="rcp")
                    for j in range(2):
                        jj = 2 * p + j
                        es = esp.tile([128, R], F16, tag="es")
                        att = esp.tile([128, R], F16, tag="att")
                        with tc.high_priority():
                            nc.scalar.activation(es[:isz, 0:R], sp[:, j, :],
                                                 mybir.ActivationFunctionType.Exp,
                                                 bias=negmax[:, j:j + 1],
                                                 scale=1.0,
                                                 accum_out=sums[:, j:j + 1])
                            nc.vector.reciprocal(rcp[:, j:j + 1],
                                                 sums[:, j:j + 1])
                            # normalize while atten is still i-partitioned
                            nc.vector.tensor_scalar_mul(att[:isz, :],
                                                        es[:isz, :],
                                                        rcp[:, j:j + 1])

                        # transpose atten -> attenT[r, i-slice] on the PE
                        # (transpose-mode matmul against identity); accumulate
                        # the full [r, 312] attenT in SBUF across i-tiles
                        for kr, (r0, rs) in enumerate(KR_TILES):
                            tp = tpsum.tile([rs, isz], F16, tag="tp",
                                            name=f"tp{kr}")
                            with tc.high_priority():
                                nc.tensor.transpose(tp[:], att[:isz, r0:r0 + rs],
                                                    ident_t[0:isz, 0:isz])
                                nc.vector.tensor_copy(
                                    esT_full[jj][kr][:, i0:i0 + isz], tp[:])

            # attend (transposed output): outT[f, i] = vfT.T @ attenT,
            # M=f (16 exact tiles), N=i=312 -- no tile waste
            for jj in range(2 * PW):
                b = 2 * half * PW + jj
                otf = outp.tile([128, FT, I], F16, tag=f"otf{jj % 2}",
                                name=f"otf{jj % 2}")
                for mf in range(FT):
                    op_ = opsum.tile([128, I], F32, tag="op", name="op")
                    for kr, (r0, rs) in enumerate(KR_TILES):
                        nc.tensor.matmul(
                            op_[:], vft_t[(jj, kr)][:, mf * 128:(mf + 1) * 128],
                            esT_full[jj][kr][:],
                            start=(kr == 0), stop=(kr == 1))
                    if mf % 2 == 0:
                        nc.scalar.copy(otf[:, mf, :], op_[:])
                    else:
                        nc.vector.tensor_copy(otf[:, mf, :], op_[:])
                for c in range(4):
                    nc.sync.dma_start(out[b, :, 4 * c:4 * (c + 1), :],
                                      otf[:, 4 * c:4 * (c + 1), :])


def _get_program(reps=1):
    key = ("nc", reps)
    if key in _CACHE:
        return _CACHE[key]
    nc = bacc.Bacc("TRN2", target_bir_lowering=False, debug=False,
                   num_devices=NCORES)
    wa_d = nc.dram_tensor("walpha", [V, F], F16, kind="ExternalInput")
    vt_d = nc.dram_tensor("vt", [V, I], F16, kind="ExternalInput")
    vf_d = nc.dram_tensor("vf", [NPAIR, 128, FT, 2 * R], F16,
                          kind="ExternalInput")
    vft_d = nc.dram_tensor("vft", [BL, R, F], F16, kind="ExternalInput")
    id_d = nc.dram_tensor("ident", [128, 128], F16, kind="ExternalInput")
    out_d = nc.dram_tensor("out", [BL, 128, FT, I], F16,
                           kind="ExternalOutput")

    with tile.TileContext(nc) as tc, ExitStack() as ctx:
        _build_body(nc, tc, ctx, wa_d.ap(), vt_d.ap(), vf_d.ap(),
                    vft_d.ap(), id_d.ap(), out_d.ap(), reps)
    nc.compile()
    _CACHE[key] = nc
    return nc


def _prep_inputs(visual_features, v, W_alpha):
    vf = np.asarray(visual_features, dtype=np.float32)
    v = np.asarray(v, dtype=np.float32)
    W = np.asarray(W_alpha, dtype=np.float32)

    walpha16 = np.ascontiguousarray(W).astype(np.float16)          # [V, F]
    vt16 = np.ascontiguousarray(v.T).astype(np.float16)            # [V, I]
    # [b, f, r] -> [bp, p=128, t=16, j*196+r]: batch-paired, per-partition
    # contiguous DMA layout
    vf16 = np.ascontiguousarray(
        vf.reshape(B // 2, 2, FT, 128, R).transpose(0, 3, 2, 1, 4)
        .reshape(B // 2, 128, FT, 2 * R)).astype(np.float16)
    vft16 = np.ascontiguousarray(vf.transpose(0, 2, 1)).astype(np.float16)

    in_maps = []
    for c in range(NCORES):
        in_maps.append({
            "walpha": walpha16,
            "vt": vt16,
            "ident": np.eye(128, dtype=np.float16),
            "vf": np.ascontiguousarray(vf16[c * NPAIR:(c + 1) * NPAIR]),
            "vft": np.ascontiguousarray(vft16[c * BL:(c + 1) * BL]),
        })
    return in_maps


def kernel(visual_features, v, W_alpha):
    nc = _get_program()
    in_maps = _prep_inputs(visual_features, v, W_alpha)
    res = None
    for attempt in range(3):
        try:
            res = bass_utils.run_bass_kernel_spmd(
                nc, in_maps, core_ids=list(range(NCORES)))
            break
        except Exception:
            # transient NRT_EXEC_UNIT_UNRECOVERABLE wedges have been seen on
            # this fabric; a re-dispatch typically succeeds
            if attempt == 2:
                raise
    outs = [res.results[c]["out"] for c in range(NCORES)]
    buf = np.concatenate(outs, axis=0)          # [B, p=128, t=16, I]
    full = buf.transpose(0, 3, 2, 1).reshape(B, I, F)   # f = t*128 + p
    return np.ascontiguousarray(full).astype(np.float32)



# revision 19
# speedup vs baseline: 1.2263x; 1.2263x over previous
"""Trainium2 Bass kernel for attribute visual attention.

Computes, for each batch b:
    q      = v @ W_alpha                  # [i, f]
    scores = q @ vf[b]                    # [i, r]
    atten  = softmax(scores, axis=r)
    out[b] = atten @ vf[b].T              # [i, f]

Sharding: data-parallel over batch b across 8 NeuronCores (8 batches per
core); v / W_alpha replicated. All matmuls run in fp16 (full PE rate on
TRN2) with fp32 PSUM accumulation; softmax statistics in fp32.

Layout notes:
- The attend matmul contracts over r, which must live on SBUF partitions
  for both operands; the host passes visual_features twice — [f, r] for
  the scores matmul and pre-transposed [r, f] for the attend matmul. The
  small e = exp(scores - max) matrix is transposed on-chip on the PE.
- Batches are processed in PAIRS for the scores matmul (rhs = two
  batches side by side, N=392): halves the number of PE instructions and
  stationary-weight loads.
- Bulk HBM traffic uses SWDGE (gpsimd); weights/identity/outputs use
  HWDGE. DMA emission follows consumption order (weights, then per wave
  vf -> vft) with no blanket priority overrides, so the tile scheduler's
  priority heap preserves it; the PE is then fed continuously, which also
  keeps the DVFS p-state at full clock (an idle gap > ~4 us resets the
  ramp and the next matmul burst runs at the slow p-state).
- Softmax normalization is applied while atten is i-partitioned; output
  store DMAs are issued per 4-f-tile chunk as soon as the chunk's
  PSUM->SBUF copies land, shortening the drain tail.
"""

import numpy as np
from contextlib import ExitStack

import concourse.bass as bass
import concourse.tile as tile
import concourse.bass_utils as bass_utils
from concourse import bacc, mybir

# Problem shapes (hardcoded per contest contract).
B, F, R, I, V = 64, 2048, 196, 312, 300
NCORES = 8
BL = B // NCORES          # 8 batches per core
NPAIR = BL // 2           # 4 batch-pairs per core
FT = F // 128             # 16 f-tiles
I_TILES = ((0, 128), (128, 128), (256, 56))
KV_TILES = ((0, 128), (128, 128), (256, 44))    # v=300
KR_TILES = ((0, 128), (128, 68))                # r=196
WARMUP = 75               # junk matmuls: finish the clock ramp AND bridge the
                          # weight-DMA window so phase 0 starts at full clock

F16 = mybir.dt.float16
F32 = mybir.dt.float32

_CACHE = {}


def _build_body(nc, tc, ctx, wa, vt, vf, vft, ident, out, reps):
    # Streaming input pools FIRST so their SBUF ranges are disjoint from the
    # phase-0 weight pool: otherwise wave-0 loads inherit an address-reuse
    # dependency on the end of phase 0 and the DMA pipeline stalls ~17 us.
    vfp = ctx.enter_context(tc.tile_pool(name="vf", bufs=3))
    vftp = ctx.enter_context(tc.tile_pool(name="vft", bufs=3))
    qtp = ctx.enter_context(tc.tile_pool(name="qt", bufs=1))
    ident_t = qtp.tile([128, 128], F16, tag="ident", name="ident")
    nc.sync.dma_start(ident_t[:], ident[:])

    # PE warm-up on a memset tile (no DMA dependency): junk matmuls from
    # ~t=0 so the clock ramp completes while the weight loads are in flight
    junk_t = qtp.tile([128, 128], F16, tag="junk", name="junk")
    nc.vector.memset(junk_t[:], 0.25)
    with tc.tile_pool(name="wupsum", bufs=1, space=bass.MemorySpace.PSUM) as wup:
        wu = wup.tile([128, 128], F32, tag="wu", name="wu")
        for w in range(WARMUP):
            nc.tensor.matmul(wu[:], junk_t[:], junk_t[:],
                             start=(w == 0), stop=(w == WARMUP - 1))

    # Phase-1 PSUM pools allocated before qpsum so the scores accumulators
    # get banks disjoint from phase 0's; qpsum's banks are recycled by the
    # attend accumulators (opsum), whose first write comes well after the
    # last phase-0 read.
    spsum = ctx.enter_context(
        tc.tile_pool(name="spsum", bufs=3, space=bass.MemorySpace.PSUM))
    tpsum = ctx.enter_context(
        tc.tile_pool(name="tpsum", bufs=1, space=bass.MemorySpace.PSUM))

    # ---- Phase 0: qT[f, i] = (v @ W_alpha).T via lhsT=W_alpha, rhs=v.T ----
    # Weight DMAs in k-major order (vt, then per-k wa chunks) so the
    # mf-loop's k=0 matmuls unblock after ~half the weight bytes.
    qt_t = []
    with tc.tile_pool(name="const", bufs=1) as const, \
         tc.tile_pool(name="qpsum", bufs=4, space=bass.MemorySpace.PSUM) as qpsum:
        vt_t, wa_t = [], []
        for k, (v0, vs) in enumerate(KV_TILES):
            t = const.tile([vs, I], F16, tag=f"vt{k}")
            nc.sync.dma_start(t[:], vt[v0:v0 + vs, :])
            vt_t.append(t)
        for k, (v0, vs) in enumerate(KV_TILES):
            wa_t.append(const.tile([vs, F], F16, tag=f"wa{k}", name=f"wa{k}"))
        for c in range(2):
            for k, (v0, vs) in enumerate(KV_TILES):
                nc.sync.dma_start(wa_t[k][:, c * 1024:(c + 1) * 1024],
                                  wa[v0:v0 + vs, c * 1024:(c + 1) * 1024])

        for mf in range(FT):
            qp = qpsum.tile([128, I], F32, tag="qp")
            for k, (v0, vs) in enumerate(KV_TILES):
                nc.tensor.matmul(qp[:], wa_t[k][:, mf * 128:(mf + 1) * 128],
                                 vt_t[k][:], start=(k == 0), stop=(k == 2))
            q = qtp.tile([128, I], F16, tag=f"qt{mf}")
            if mf % 2 == 0:
                nc.scalar.copy(q[:], qp[:])
            else:
                nc.vector.tensor_copy(q[:], qp[:])
            qt_t.append(q)

    # Hold the SWDGE bulk queue off the shared DMA engines while the
    # (phase-0-critical) weight DMAs stream in: one long Pool-engine memset
    # emitted ahead of the first prep. Both are ready at t=0, so the
    # scheduler's priority heap keeps the memset first; the weights then get
    # the DMA engines exclusively for the first ~5 us.
    dhold = qtp.tile([128, 4800], F16, tag="dhold", name="dhold")
    nc.gpsimd.memset(dhold[:], 0.0)

    # ---- Phase 1: per batch-pair attention ----
    esp = ctx.enter_context(tc.tile_pool(name="es", bufs=6))
    attp = ctx.enter_context(tc.tile_pool(name="atT", bufs=3))
    outp = ctx.enter_context(tc.tile_pool(name="out", bufs=2))
    stat = ctx.enter_context(tc.tile_pool(name="stat", bufs=8))
    opsum = ctx.enter_context(
        tc.tile_pool(name="opsum", bufs=4, space=bass.MemorySpace.PSUM))

    for rep in range(reps):
        for half in range(NPAIR):
            if half > 0:
                # PSUM-free PE activity across any wave-boundary wait:
                # standalone weight loads keep the clock-ramp monitor fed
                for _ in range(10):
                    nc.tensor.ldweights(ident_t[:])
            # vf pair tile [128, t, j*196+r], then vft per batch — emitted
            # in consumption order on the SWDGE queue
            bp = half
            vf_t = vfp.tile([128, FT, 2 * R], F16, tag="vf", name="vf0")
            for c in range(4):
                nc.gpsimd.dma_start(vf_t[:, 4 * c:4 * (c + 1), :],
                                    vf[bp, :, 4 * c:4 * (c + 1), :])
            vft_t = {}
            for j in range(2):
                b = 2 * bp + j
                for kr, (r0, rs) in enumerate(KR_TILES):
                    vv = vftp.tile([rs, F], F16, tag=f"vft{kr}{j}",
                                   name=f"vft{kr}{j}")
                    nc.gpsimd.dma_start(vv[:], vft[b, r0:r0 + rs, :])
                    vft_t[(j, kr)] = vv

            esT_full = [
                [attp.tile([rs, I], F16, tag=f"esT{kr}{j}",
                           name=f"esT{kr}{j}")
                 for kr, (r0, rs) in enumerate(KR_TILES)]
                for j in range(2)]
            for mi, (i0, isz) in enumerate(I_TILES):
                sp = spsum.tile([isz, 2, R], F32, tag="sp", name="sp")
                for kf in range(FT):
                    nc.tensor.matmul(
                        sp[:], qt_t[kf][:, i0:i0 + isz],
                        vf_t[:, kf, :].rearrange("p (j r) -> p j r", j=2),
                        start=(kf == 0), stop=(kf == FT - 1))

                negmax = stat.tile([isz, 2], F32, tag="negmax")
                with tc.high_priority():
                    nc.vector.tensor_reduce(negmax[:], sp[:],
                                            axis=mybir.AxisListType.X,
                                            op=mybir.AluOpType.max, negate=True)
                sums = stat.tile([isz, 2], F32, tag="sums")
                rcp = stat.tile([isz, 2], F32, tag="rcp")
                for j in range(2):
                    es = esp.tile([128, R], F16, tag="es")
                    att = esp.tile([128, R], F16, tag="att")
                    with tc.high_priority():
                        nc.scalar.activation(es[:isz, 0:R], sp[:, j, :],
                                             mybir.ActivationFunctionType.Exp,
                                             bias=negmax[:, j:j + 1],
                                             scale=1.0,
                                             accum_out=sums[:, j:j + 1])
                        nc.vector.reciprocal(rcp[:, j:j + 1],
                                             sums[:, j:j + 1])
                        # normalize while atten is still i-partitioned
                        nc.vector.tensor_scalar_mul(att[:isz, :],
                                                    es[:isz, :],
                                                    rcp[:, j:j + 1])

                    # transpose atten -> attenT[r, i-slice] on the PE
                    # (transpose-mode matmul against identity); accumulate
                    # the full [r, 312] attenT in SBUF across i-tiles
                    for kr, (r0, rs) in enumerate(KR_TILES):
                        tp = tpsum.tile([rs, isz], F16, tag="tp",
                                        name=f"tp{kr}")
                        with tc.high_priority():
                            nc.tensor.transpose(tp[:], att[:isz, r0:r0 + rs],
                                                ident_t[0:isz, 0:isz])
                            nc.vector.tensor_copy(
                                esT_full[j][kr][:, i0:i0 + isz], tp[:])

            # attend (transposed output): outT[f, i] = vfT.T @ attenT,
            # M=f (16 exact tiles), N=i=312 -- no tile waste. Output DMA
            # per 4-f-tile chunk as soon as its copies land. On the last
            # wave: i-sliced rhs (same PE time; lets attend start before
            # the mi=2 softmax lands, since no next-wave scores hide that
            # latency) and 2-tile output chunks (shorter drain tail).
            last_wave = (rep == reps - 1 and half == NPAIR - 1)
            for j in range(2):
                b = 2 * half + j
                otf = outp.tile([128, FT, I], F16, tag=f"otf{j}",
                                name=f"otf{j}")
                for mf in range(FT):
                    op_ = opsum.tile([128, I], F32, tag="op", name="op")
                    for kr, (r0, rs) in enumerate(KR_TILES):
                        nc.tensor.matmul(
                            op_[:],
                            vft_t[(j, kr)][:, mf * 128:(mf + 1) * 128],
                            esT_full[j][kr][:],
                            start=(kr == 0), stop=(kr == 1))
                    if mf % 2 == 0:
                        nc.scalar.copy(otf[:, mf, :], op_[:])
                    else:
                        nc.vector.tensor_copy(otf[:, mf, :], op_[:])
                    if last_wave and j == 1 and mf >= 12:
                        if mf % 2 == 1:
                            c = mf // 2
                            nc.sync.dma_start(out[b, :, 2 * c:2 * (c + 1), :],
                                              otf[:, 2 * c:2 * (c + 1), :])
                    elif mf % 4 == 3:
                        c = mf // 4
                        nc.sync.dma_start(out[b, :, 4 * c:4 * (c + 1), :],
                                          otf[:, 4 * c:4 * (c + 1), :])


def _get_program(reps=1):
    key = ("nc", reps)
    if key in _CACHE:
        return _CACHE[key]
    nc = bacc.Bacc("TRN2", target_bir_lowering=False, debug=False,
                   num_devices=NCORES)
    wa_d = nc.dram_tensor("walpha", [V, F], F16, kind="ExternalInput")
    vt_d = nc.dram_tensor("vt", [V, I], F16, kind="ExternalInput")
    vf_d = nc.dram_tensor("vf", [NPAIR, 128, FT, 2 * R], F16,
                          kind="ExternalInput")
    vft_d = nc.dram_tensor("vft", [BL, R, F], F16, kind="ExternalInput")
    id_d = nc.dram_tensor("ident", [128, 128], F16, kind="ExternalInput")
    out_d = nc.dram_tensor("out", [BL, 128, FT, I], F16,
                           kind="ExternalOutput")

    with tile.TileContext(nc) as tc, ExitStack() as ctx:
        _build_body(nc, tc, ctx, wa_d.ap(), vt_d.ap(), vf_d.ap(),
                    vft_d.ap(), id_d.ap(), out_d.ap(), reps)
    nc.compile()
    _CACHE[key] = nc
    return nc


def _prep_inputs(visual_features, v, W_alpha):
    vf = np.asarray(visual_features, dtype=np.float32)
    v = np.asarray(v, dtype=np.float32)
    W = np.asarray(W_alpha, dtype=np.float32)

    walpha16 = np.ascontiguousarray(W).astype(np.float16)          # [V, F]
    vt16 = np.ascontiguousarray(v.T).astype(np.float16)            # [V, I]
    # [b, f, r] -> [bp, p=128, t=16, j*196+r]: batch-paired, per-partition
    # contiguous DMA layout
    vf16 = np.ascontiguousarray(
        vf.reshape(B // 2, 2, FT, 128, R).transpose(0, 3, 2, 1, 4)
        .reshape(B // 2, 128, FT, 2 * R)).astype(np.float16)
    vft16 = np.ascontiguousarray(vf.transpose(0, 2, 1)).astype(np.float16)

    in_maps = []
    for c in range(NCORES):
        in_maps.append({
            "walpha": walpha16,
            "vt": vt16,
            "ident": np.eye(128, dtype=np.float16),
            "vf": np.ascontiguousarray(vf16[c * NPAIR:(c + 1) * NPAIR]),
            "vft": np.ascontiguousarray(vft16[c * BL:(c + 1) * BL]),
        })
    return in_maps


def kernel(visual_features, v, W_alpha):
    nc = _get_program()
    in_maps = _prep_inputs(visual_features, v, W_alpha)
    res = None
    for attempt in range(3):
        try:
            res = bass_utils.run_bass_kernel_spmd(
                nc, in_maps, core_ids=list(range(NCORES)))
            break
        except Exception:
            # transient NRT_EXEC_UNIT_UNRECOVERABLE wedges have been seen on
            # this fabric; a re-dispatch typically succeeds
            if attempt == 2:
                raise
    outs = [res.results[c]["out"] for c in range(NCORES)]
    buf = np.concatenate(outs, axis=0)          # [B, p=128, t=16, I]
    full = buf.transpose(0, 3, 2, 1).reshape(B, I, F)   # f = t*128 + p
    return np.ascontiguousarray(full).astype(np.float32)


# revision 28
# speedup vs baseline: 1.2714x; 1.0368x over previous
"""Trainium2 Bass kernel for attribute visual attention.

Computes, for each batch b:
    q      = v @ W_alpha                  # [i, f]
    scores = q @ vf[b]                    # [i, r]
    atten  = softmax(scores, axis=r)
    out[b] = atten @ vf[b].T              # [i, f]

Sharding: data-parallel over batch b across 8 NeuronCores (8 batches per
core); v / W_alpha replicated. All matmuls run in fp16 (full PE rate on
TRN2) with fp32 PSUM accumulation; softmax statistics in fp32.

Layout notes:
- The attend matmul contracts over r, which must live on SBUF partitions
  for both operands; the host passes visual_features twice — [f, r] for
  the scores matmul and pre-transposed [r, f] for the attend matmul. The
  small e = exp(scores - max) matrix is transposed on-chip on the PE.
- Batches are processed in PAIRS for the scores matmul (rhs = two
  batches side by side, N=392): halves the number of PE instructions and
  stationary-weight loads.
- Bulk HBM traffic uses SWDGE (gpsimd); weights/identity/outputs use
  HWDGE. DMA emission follows consumption order (weights, then per wave
  vf -> vft) with no blanket priority overrides, so the tile scheduler's
  priority heap preserves it; the PE is then fed continuously, which also
  keeps the DVFS p-state at full clock (an idle gap > ~4 us resets the
  ramp and the next matmul burst runs at the slow p-state).
- Softmax normalization is applied while atten is i-partitioned; output
  store DMAs are issued per 4-f-tile chunk as soon as the chunk's
  PSUM->SBUF copies land, shortening the drain tail.
"""

import numpy as np
from contextlib import ExitStack

import concourse.bass as bass
import concourse.tile as tile
import concourse.bass_utils as bass_utils
from concourse import bacc, mybir

# Problem shapes (hardcoded per contest contract).
B, F, R, I, V = 64, 2048, 196, 312, 300
NCORES = 8
BL = B // NCORES          # 8 batches per core
NPAIR = BL // 2           # 4 batch-pairs per core
FT = F // 128             # 16 f-tiles
I_TILES = ((0, 128), (128, 128), (256, 56))
KV_TILES = ((0, 128), (128, 128), (256, 44))    # v=300
KR_TILES = ((0, 128), (128, 68))                # r=196
WARMUP = 60               # junk matmuls: finish the clock ramp AND bridge the
                          # weight-DMA window so phase 0 starts at full clock

F16 = mybir.dt.float16
F32 = mybir.dt.float32

_CACHE = {}


def _build_body(nc, tc, ctx, wa, vt, vf, vft, ident, out, reps):
    # Streaming input pools FIRST so their SBUF ranges are disjoint from the
    # phase-0 weight pool: otherwise wave-0 loads inherit an address-reuse
    # dependency on the end of phase 0 and the DMA pipeline stalls ~17 us.
    vfp = ctx.enter_context(tc.tile_pool(name="vf", bufs=3))
    vftp = ctx.enter_context(tc.tile_pool(name="vft", bufs=3))
    qtp = ctx.enter_context(tc.tile_pool(name="qt", bufs=1))
    ident_t = qtp.tile([128, 128], F16, tag="ident", name="ident")

    # PE warm-up on a memset tile (no DMA dependency): junk matmuls from
    # ~t=0 so the clock ramp completes while the weight loads are in flight
    junk_t = qtp.tile([128, 128], F16, tag="junk", name="junk")
    nc.gpsimd.memset(junk_t[:], 0.25)
    with tc.tile_pool(name="wupsum", bufs=1, space=bass.MemorySpace.PSUM) as wup:
        wu = wup.tile([128, 128], F32, tag="wu", name="wu")
        for w in range(WARMUP):
            nc.tensor.matmul(wu[:], junk_t[:], junk_t[:],
                             start=(w == 0), stop=(w == WARMUP - 1))

    # Phase-1 PSUM pools allocated before qpsum so the scores accumulators
    # get banks disjoint from phase 0's; qpsum's banks are recycled by the
    # attend accumulators (opsum), whose first write comes well after the
    # last phase-0 read.
    spsum = ctx.enter_context(
        tc.tile_pool(name="spsum", bufs=2, space=bass.MemorySpace.PSUM))
    tpsum = ctx.enter_context(
        tc.tile_pool(name="tpsum", bufs=1, space=bass.MemorySpace.PSUM))

    # ---- Phase 0: qT[f, i] = (v @ W_alpha).T via lhsT=W_alpha, rhs=v.T ----
    # Weight DMAs in k-major order (vt, then per-k wa chunks) so the
    # mf-loop's k=0 matmuls unblock after ~half the weight bytes.
    qt_t = []
    with tc.tile_pool(name="const", bufs=1) as const, \
         tc.tile_pool(name="qpsum", bufs=4, space=bass.MemorySpace.PSUM) as qpsum:
        vt_t, wa_t = [], []
        for k, (v0, vs) in enumerate(KV_TILES):
            t = const.tile([vs, I], F16, tag=f"vt{k}")
            nc.sync.dma_start(t[:], vt[v0:v0 + vs, :])
            vt_t.append(t)
        for k, (v0, vs) in enumerate(KV_TILES):
            wa_t.append(const.tile([vs, F], F16, tag=f"wa{k}", name=f"wa{k}"))
        for c in range(2):
            for k, (v0, vs) in enumerate(KV_TILES):
                nc.sync.dma_start(wa_t[k][:, c * 1024:(c + 1) * 1024],
                                  wa[v0:v0 + vs, c * 1024:(c + 1) * 1024])
        # identity (for the PE transposes, first needed at scores time)
        # loads after the weights so it doesn't crowd the HWDGE head
        nc.sync.dma_start(ident_t[:], ident[:])

        for mf in range(FT):
            qp = qpsum.tile([128, I], F32, tag="qp")
            for k, (v0, vs) in enumerate(KV_TILES):
                nc.tensor.matmul(qp[:], wa_t[k][:, mf * 128:(mf + 1) * 128],
                                 vt_t[k][:], start=(k == 0), stop=(k == 2))
            q = qtp.tile([128, I], F16, tag=f"qt{mf}")
            if mf % 2 == 0:
                nc.scalar.copy(q[:], qp[:])
            else:
                nc.vector.tensor_copy(q[:], qp[:])
            qt_t.append(q)

    # Hold the SWDGE bulk queue off the shared DMA engines while the
    # (phase-0-critical) weight DMAs stream in: one long Pool-engine memset
    # emitted ahead of the first prep. Both are ready at t=0, so the
    # scheduler's priority heap keeps the memset first; the weights then get
    # the DMA engines exclusively for the first ~5 us.
    dhold = qtp.tile([128, 4200], F16, tag="dhold", name="dhold")
    nc.gpsimd.memset(dhold[:], 0.0)

    # ---- Phase 1: per batch-pair attention ----
    esp = ctx.enter_context(tc.tile_pool(name="es", bufs=6))
    attp = ctx.enter_context(tc.tile_pool(name="atT", bufs=3))
    outp = ctx.enter_context(tc.tile_pool(name="out", bufs=2))
    stat = ctx.enter_context(tc.tile_pool(name="stat", bufs=8))
    opsum = ctx.enter_context(
        tc.tile_pool(name="opsum", bufs=4, space=bass.MemorySpace.PSUM))

    def emit_scores(half):
        # vf pair tile [128, t, j*196+r], then vft per batch — emitted
        # in consumption order on the SWDGE queue
        vf_t = vfp.tile([128, FT, 2 * R], F16, tag="vf", name="vf0")
        for c in range(4):
            nc.gpsimd.dma_start(vf_t[:, 4 * c:4 * (c + 1), :],
                                vf[half, :, 4 * c:4 * (c + 1), :])
        vft_t = {}
        for j in range(2):
            b = 2 * half + j
            for kr, (r0, rs) in enumerate(KR_TILES):
                vv = vftp.tile([rs, F], F16, tag=f"vft{kr}{j}",
                               name=f"vft{kr}{j}")
                nc.gpsimd.dma_start(vv[:], vft[b, r0:r0 + rs, :])
                vft_t[(j, kr)] = vv

        esT_full = [
            [attp.tile([rs, I], F16, tag=f"esT{kr}{j}",
                       name=f"esT{kr}{j}")
             for kr, (r0, rs) in enumerate(KR_TILES)]
            for j in range(2)]
        for mi, (i0, isz) in enumerate(I_TILES):
            sp = spsum.tile([isz, 2, R], F32, tag="sp", name="sp")
            for kf in range(FT):
                nc.tensor.matmul(
                    sp[:], qt_t[kf][:, i0:i0 + isz],
                    vf_t[:, kf, :].rearrange("p (j r) -> p j r", j=2),
                    start=(kf == 0), stop=(kf == FT - 1))

            negmax = stat.tile([isz, 2], F32, tag="negmax")
            with tc.high_priority():
                nc.vector.tensor_reduce(negmax[:], sp[:],
                                        axis=mybir.AxisListType.X,
                                        op=mybir.AluOpType.max, negate=True)
            sums = stat.tile([isz, 2], F32, tag="sums")
            rcp = stat.tile([isz, 2], F32, tag="rcp")
            for j in range(2):
                es = esp.tile([128, R], F16, tag="es")
                att = esp.tile([128, R], F16, tag="att")
                with tc.high_priority():
                    nc.scalar.activation(es[:isz, 0:R], sp[:, j, :],
                                         mybir.ActivationFunctionType.Exp,
                                         bias=negmax[:, j:j + 1],
                                         scale=1.0,
                                         accum_out=sums[:, j:j + 1])
                    nc.vector.reciprocal(rcp[:, j:j + 1],
                                         sums[:, j:j + 1])
                    # normalize while atten is still i-partitioned
                    nc.vector.tensor_scalar_mul(att[:isz, :],
                                                es[:isz, :],
                                                rcp[:, j:j + 1])

                # transpose atten -> attenT[r, i-slice] on the PE
                # (transpose-mode matmul against identity); accumulate
                # the full [r, 312] attenT in SBUF across i-tiles
                for kr, (r0, rs) in enumerate(KR_TILES):
                    tp = tpsum.tile([rs, isz], F16, tag=f"tp{kr}",
                                    name=f"tp{kr}")
                    with tc.high_priority():
                        nc.tensor.transpose(tp[:], att[:isz, r0:r0 + rs],
                                            ident_t[0:isz, 0:isz])
                        nc.vector.tensor_copy(
                            esT_full[j][kr][:, i0:i0 + isz], tp[:])
        return vft_t, esT_full

    def emit_attend(half, vft_t, esT_full, last_wave):
        # attend (transposed output): outT[f, i] = vfT.T @ attenT,
        # M=f (16 exact tiles), N=i=312 -- no tile waste. Output DMA
        # per 4-f-tile chunk as soon as its copies land; the final batch
        # stores its last chunk as two 2-tile pieces (shorter drain tail).
        for j in range(2):
            b = 2 * half + j
            otf = outp.tile([128, FT, I], F16, tag=f"otf{j}",
                            name=f"otf{j}")
            for mf in range(FT):
                op_ = opsum.tile([128, I], F32, tag="op", name="op")
                for kr, (r0, rs) in enumerate(KR_TILES):
                    nc.tensor.matmul(
                        op_[:],
                        vft_t[(j, kr)][:, mf * 128:(mf + 1) * 128],
                        esT_full[j][kr][:],
                        start=(kr == 0), stop=(kr == 1))
                if mf % 2 == 0:
                    nc.scalar.copy(otf[:, mf, :], op_[:])
                else:
                    nc.vector.tensor_copy(otf[:, mf, :], op_[:])
                if last_wave and j == 1 and mf >= 12:
                    if mf == 13:
                        nc.sync.dma_start(out[b, :, 12:14, :],
                                          otf[:, 12:14, :])
                    elif mf >= 14:
                        nc.sync.dma_start(out[b, :, mf:mf + 1, :],
                                          otf[:, mf:mf + 1, :])
                elif mf % 4 == 3:
                    c = mf // 4
                    nc.sync.dma_start(out[b, :, 4 * c:4 * (c + 1), :],
                                      otf[:, 4 * c:4 * (c + 1), :])

    # Software pipeline: attend for pair p is emitted after the scores of
    # pair p+1, so the scores->softmax->transpose latency of p is hidden
    # under p+1's score matmuls instead of stalling the attend ldweights.
    pending = None
    for rep in range(reps):
        for half in range(NPAIR):
            if rep + half > 0:
                # PSUM-free PE activity across any wave-boundary wait:
                # standalone weight loads keep the clock-ramp monitor fed
                for _ in range(10):
                    nc.tensor.ldweights(ident_t[:])
            state = emit_scores(half)
            if pending is not None:
                emit_attend(*pending, last_wave=False)
            pending = (half,) + state
    emit_attend(*pending, last_wave=True)


def _get_program(reps=1):
    key = ("nc", reps)
    if key in _CACHE:
        return _CACHE[key]
    nc = bacc.Bacc("TRN2", target_bir_lowering=False, debug=False,
                   num_devices=NCORES)
    wa_d = nc.dram_tensor("walpha", [V, F], F16, kind="ExternalInput")
    vt_d = nc.dram_tensor("vt", [V, I], F16, kind="ExternalInput")
    vf_d = nc.dram_tensor("vf", [NPAIR, 128, FT, 2 * R], F16,
                          kind="ExternalInput")
    vft_d = nc.dram_tensor("vft", [BL, R, F], F16, kind="ExternalInput")
    id_d = nc.dram_tensor("ident", [128, 128], F16, kind="ExternalInput")
    out_d = nc.dram_tensor("out", [BL, 128, FT, I], F16,
                           kind="ExternalOutput")

    with tile.TileContext(nc) as tc, ExitStack() as ctx:
        _build_body(nc, tc, ctx, wa_d.ap(), vt_d.ap(), vf_d.ap(),
                    vft_d.ap(), id_d.ap(), out_d.ap(), reps)
    nc.compile()
    _CACHE[key] = nc
    return nc


def _prep_inputs(visual_features, v, W_alpha):
    vf = np.asarray(visual_features, dtype=np.float32)
    v = np.asarray(v, dtype=np.float32)
    W = np.asarray(W_alpha, dtype=np.float32)

    walpha16 = np.ascontiguousarray(W).astype(np.float16)          # [V, F]
    vt16 = np.ascontiguousarray(v.T).astype(np.float16)            # [V, I]
    # [b, f, r] -> [bp, p=128, t=16, j*196+r]: batch-paired, per-partition
    # contiguous DMA layout
    vf16 = np.ascontiguousarray(
        vf.reshape(B // 2, 2, FT, 128, R).transpose(0, 3, 2, 1, 4)
        .reshape(B // 2, 128, FT, 2 * R)).astype(np.float16)
    vft16 = np.ascontiguousarray(vf.transpose(0, 2, 1)).astype(np.float16)

    in_maps = []
    for c in range(NCORES):
        in_maps.append({
            "walpha": walpha16,
            "vt": vt16,
            "ident": np.eye(128, dtype=np.float16),
            "vf": np.ascontiguousarray(vf16[c * NPAIR:(c + 1) * NPAIR]),
            "vft": np.ascontiguousarray(vft16[c * BL:(c + 1) * BL]),
        })
    return in_maps


def kernel(visual_features, v, W_alpha):
    nc = _get_program()
    in_maps = _prep_inputs(visual_features, v, W_alpha)
    res = None
    for attempt in range(3):
        try:
            res = bass_utils.run_bass_kernel_spmd(
                nc, in_maps, core_ids=list(range(NCORES)))
            break
        except Exception:
            # transient NRT_EXEC_UNIT_UNRECOVERABLE wedges have been seen on
            # this fabric; a re-dispatch typically succeeds
            if attempt == 2:
                raise
    outs = [res.results[c]["out"] for c in range(NCORES)]
    buf = np.concatenate(outs, axis=0)          # [B, p=128, t=16, I]
    full = buf.transpose(0, 3, 2, 1).reshape(B, I, F)   # f = t*128 + p
    return np.ascontiguousarray(full).astype(np.float32)


# revision 49
# speedup vs baseline: 1.2788x; 1.0058x over previous
"""Trainium2 Bass kernel for attribute visual attention.

Computes, for each batch b:
    q      = v @ W_alpha                  # [i, f]
    scores = q @ vf[b]                    # [i, r]
    atten  = softmax(scores, axis=r)
    out[b] = atten @ vf[b].T              # [i, f]

Sharding: data-parallel over batch b across 8 NeuronCores (8 batches per
core); v / W_alpha replicated. All matmuls run in fp16 (full PE rate on
TRN2) with fp32 PSUM accumulation; softmax statistics in fp32.

Layout notes:
- The attend matmul contracts over r, which must live on SBUF partitions
  for both operands; the host passes visual_features twice — [f, r] for
  the scores matmul and pre-transposed [r, f] for the attend matmul. The
  small e = exp(scores - max) matrix is transposed on-chip on the PE.
- Batches are processed in PAIRS for the scores matmul (rhs = two
  batches side by side, N=392): halves the number of PE instructions and
  stationary-weight loads.
- Bulk HBM traffic uses SWDGE (gpsimd); weights/identity/outputs use
  HWDGE. DMA emission follows consumption order (weights, then per wave
  vf -> vft) with no blanket priority overrides, so the tile scheduler's
  priority heap preserves it; the PE is then fed continuously, which also
  keeps the DVFS p-state at full clock (an idle gap > ~4 us resets the
  ramp and the next matmul burst runs at the slow p-state).
- Streaming pools are allocated ahead of the phase-0 weight pool so their
  SBUF ranges are disjoint (address reuse would chain wave-0 loads behind
  the end of phase 0); a long Pool-engine memset (dhold) keeps the bulk
  SWDGE queue off the shared DMA engines while the weights stream in.
- Phase 1 is software-pipelined one wave deep: the attend matmuls of pair
  p are emitted after the scores of pair p+1, hiding the per-pair
  scores -> softmax -> transpose latency chain under score matmuls.
- Softmax normalization is applied while atten is i-partitioned; output
  store DMAs are issued per 4-f-tile chunk as soon as the chunk's
  PSUM->SBUF copies land, shortening the drain tail.
"""

import numpy as np
from contextlib import ExitStack

import concourse.bass as bass
import concourse.tile as tile
import concourse.bass_utils as bass_utils
from concourse import bacc, mybir

# Problem shapes (hardcoded per contest contract).
B, F, R, I, V = 64, 2048, 196, 312, 300
NCORES = 8
BL = B // NCORES          # 8 batches per core
NPAIR = BL // 2           # 4 batch-pairs per core
FT = F // 128             # 16 f-tiles
I_TILES = ((0, 128), (128, 128), (256, 56))
KV_TILES = ((0, 128), (128, 128), (256, 44))    # v=300
KR_TILES = ((0, 128), (128, 68))                # r=196
import os as _os
WARMUP = int(_os.environ.get("K_WARMUP", "45"))
                          # junk matmuls: finish the clock ramp AND bridge the
                          # weight-DMA window so phase 0 starts at full clock
DHOLD = int(_os.environ.get("K_DHOLD", "4200"))
VFP_BUFS = int(_os.environ.get("K_VFP", "3"))
VFTP_BUFS = int(_os.environ.get("K_VFTP", "3"))
OPSUM_BUFS = int(_os.environ.get("K_OPSUM", "4"))
QPSUM_BUFS = int(_os.environ.get("K_QPSUM", "4"))

F16 = mybir.dt.float16
F32 = mybir.dt.float32

_CACHE = {}


def _build_body(nc, tc, ctx, wa, vt, vf, vft, ident, out, reps):
    # Streaming input pools FIRST so their SBUF ranges are disjoint from the
    # phase-0 weight pool: otherwise wave-0 loads inherit an address-reuse
    # dependency on the end of phase 0 and the DMA pipeline stalls ~17 us.
    vfp = ctx.enter_context(tc.tile_pool(name="vf", bufs=VFP_BUFS))
    vftp = ctx.enter_context(tc.tile_pool(name="vft", bufs=VFTP_BUFS))
    qtp = ctx.enter_context(tc.tile_pool(name="qt", bufs=1))
    ident_t = qtp.tile([128, 128], F16, tag="ident", name="ident")

    # PE warm-up on a memset tile (no DMA dependency): junk matmuls from
    # ~t=0 so the clock ramp completes while the weight loads are in flight
    junk_t = qtp.tile([128, 128], F16, tag="junk", name="junk")
    nc.vector.memset(junk_t[:], 0.25)

    with tc.tile_pool(name="wupsum", bufs=1, space=bass.MemorySpace.PSUM) as wup:
        wu = wup.tile([128, 128], F32, tag="wu", name="wu")
        for w in range(WARMUP):
            nc.tensor.matmul(wu[:], junk_t[:], junk_t[:],
                             start=(w == 0), stop=(w == WARMUP - 1))
    # PSUM-free PE activity bridging the warmup -> phase-0 weight wait: on
    # real hardware the DVFS monitor needs sustained work to hold the clock
    for _ in range(30):
        nc.tensor.ldweights(junk_t[:])

    # Phase-1 PSUM pools allocated before qpsum so the scores accumulators
    # get banks disjoint from phase 0's; qpsum's banks are recycled by the
    # attend accumulators (opsum), whose first write comes well after the
    # last phase-0 read.
    spsum = ctx.enter_context(
        tc.tile_pool(name="spsum", bufs=2, space=bass.MemorySpace.PSUM))
    tpsum = ctx.enter_context(
        tc.tile_pool(name="tpsum", bufs=1, space=bass.MemorySpace.PSUM))

    # ---- Phase 0: qT[f, i] = (v @ W_alpha).T via lhsT=W_alpha, rhs=v.T ----
    # Weight DMAs in k-major order (vt, then per-k wa chunks) so the
    # mf-loop's k=0 matmuls unblock after ~half the weight bytes.
    qt_t = []
    with tc.tile_pool(name="const", bufs=1) as const, \
         tc.tile_pool(name="qpsum", bufs=QPSUM_BUFS, space=bass.MemorySpace.PSUM) as qpsum:
        vt_t, wa_t = [], []
        for k, (v0, vs) in enumerate(KV_TILES):
            t = const.tile([vs, I], F16, tag=f"vt{k}")
            nc.sync.dma_start(t[:], vt[v0:v0 + vs, :])
            vt_t.append(t)
        for k, (v0, vs) in enumerate(KV_TILES):
            wa_t.append(const.tile([vs, F], F16, tag=f"wa{k}", name=f"wa{k}"))
        for c in range(2):
            for k, (v0, vs) in enumerate(KV_TILES):
                nc.sync.dma_start(wa_t[k][:, c * 1024:(c + 1) * 1024],
                                  wa[v0:v0 + vs, c * 1024:(c + 1) * 1024])
        # identity (for the PE transposes, first needed at scores time)
        # loads after the weights so it doesn't crowd the HWDGE head
        nc.sync.dma_start(ident_t[:], ident[:])

        for mf in range(FT):
            qp = qpsum.tile([128, I], F32, tag="qp")
            for k, (v0, vs) in enumerate(KV_TILES):
                nc.tensor.matmul(qp[:], wa_t[k][:, mf * 128:(mf + 1) * 128],
                                 vt_t[k][:], start=(k == 0), stop=(k == 2))
            q = qtp.tile([128, I], F16, tag=f"qt{mf}")
            if mf % 2 == 0:
                nc.scalar.copy(q[:], qp[:])
            else:
                nc.vector.tensor_copy(q[:], qp[:])
            qt_t.append(q)

    # Hold the SWDGE bulk queue off the shared DMA engines while the
    # (phase-0-critical) weight DMAs stream in: one long Pool-engine memset
    # emitted ahead of the first prep. Both are ready at t=0, so the
    # scheduler's priority heap keeps the memset first; the weights then get
    # the DMA engines exclusively for the first ~5 us.
    dhold = qtp.tile([128, DHOLD], F16, tag="dhold", name="dhold")
    nc.gpsimd.memset(dhold[:], 0.0)

    # ---- Phase 1: per batch-pair attention ----
    esp = ctx.enter_context(tc.tile_pool(name="es", bufs=6))
    attp = ctx.enter_context(tc.tile_pool(name="atT", bufs=3))
    outp = ctx.enter_context(tc.tile_pool(name="out", bufs=3))
    stat = ctx.enter_context(tc.tile_pool(name="stat", bufs=8))
    opsum = ctx.enter_context(
        tc.tile_pool(name="opsum", bufs=OPSUM_BUFS, space=bass.MemorySpace.PSUM))

    def emit_scores(half):
        # vf pair tile [128, t, j*196+r], then vft per batch — emitted
        # in consumption order on the SWDGE queue
        vf_t = vfp.tile([128, FT, 2 * R], F16, tag="vf", name="vf0")
        for c in range(4):
            nc.gpsimd.dma_start(vf_t[:, 4 * c:4 * (c + 1), :],
                                vf[half, :, 4 * c:4 * (c + 1), :])
        vft_t = {}
        for j in range(2):
            b = 2 * half + j
            for kr, (r0, rs) in enumerate(KR_TILES):
                vv = vftp.tile([rs, F], F16, tag=f"vft{kr}{j}",
                               name=f"vft{kr}{j}")
                nc.gpsimd.dma_start(vv[:], vft[b, r0:r0 + rs, :])
                vft_t[(j, kr)] = vv

        esT_full = [
            [attp.tile([rs, I], F16, tag=f"esT{kr}{j}",
                       name=f"esT{kr}{j}")
             for kr, (r0, rs) in enumerate(KR_TILES)]
            for j in range(2)]
        for mi, (i0, isz) in enumerate(I_TILES):
            sp = spsum.tile([isz, 2, R], F32, tag="sp", name="sp")
            for kf in range(FT):
                nc.tensor.matmul(
                    sp[:], qt_t[kf][:, i0:i0 + isz],
                    vf_t[:, kf, :].rearrange("p (j r) -> p j r", j=2),
                    start=(kf == 0), stop=(kf == FT - 1))

            negmax = stat.tile([isz, 2], F32, tag="negmax")
            with tc.high_priority():
                nc.vector.tensor_reduce(negmax[:], sp[:],
                                        axis=mybir.AxisListType.X,
                                        op=mybir.AluOpType.max, negate=True)
            sums = stat.tile([isz, 2], F32, tag="sums")
            rcp = stat.tile([isz, 2], F32, tag="rcp")
            for j in range(2):
                es = esp.tile([128, R], F16, tag="es")
                att = esp.tile([128, R], F16, tag="att")
                with tc.high_priority():
                    nc.scalar.activation(es[:isz, 0:R], sp[:, j, :],
                                         mybir.ActivationFunctionType.Exp,
                                         bias=negmax[:, j:j + 1],
                                         scale=1.0,
                                         accum_out=sums[:, j:j + 1])
                    nc.vector.reciprocal(rcp[:, j:j + 1],
                                         sums[:, j:j + 1])
                    # normalize while atten is still i-partitioned
                    nc.vector.tensor_scalar_mul(att[:isz, :],
                                                es[:isz, :],
                                                rcp[:, j:j + 1])

                # transpose atten -> attenT[r, i-slice] on the PE
                # (transpose-mode matmul against identity); accumulate
                # the full [r, 312] attenT in SBUF across i-tiles
                for kr, (r0, rs) in enumerate(KR_TILES):
                    tp = tpsum.tile([rs, isz], F16, tag=f"tp{kr}",
                                    name=f"tp{kr}")
                    with tc.high_priority():
                        nc.tensor.transpose(tp[:], att[:isz, r0:r0 + rs],
                                            ident_t[0:isz, 0:isz])
                        nc.vector.tensor_copy(
                            esT_full[j][kr][:, i0:i0 + isz], tp[:])
        return vft_t, esT_full

    def emit_attend(half, vft_t, esT_full, last_wave):
        # attend (transposed output): outT[f, i] = vfT.T @ attenT,
        # M=f (16 exact tiles), N=i=312 -- no tile waste. Output DMA
        # per 4-f-tile chunk as soon as its copies land; the final batch
        # stores its last chunk as two 2-tile pieces (shorter drain tail).
        for j in range(2):
            b = 2 * half + j
            otf = outp.tile([128, FT, I], F16, tag=f"otf{j}",
                            name=f"otf{j}")
            for mf in range(FT):
                op_ = opsum.tile([128, I], F32, tag="op", name="op")
                for kr, (r0, rs) in enumerate(KR_TILES):
                    nc.tensor.matmul(
                        op_[:],
                        vft_t[(j, kr)][:, mf * 128:(mf + 1) * 128],
                        esT_full[j][kr][:],
                        start=(kr == 0), stop=(kr == 1))
                if mf % 2 == 0:
                    nc.scalar.copy(otf[:, mf, :], op_[:])
                else:
                    nc.vector.tensor_copy(otf[:, mf, :], op_[:])
                if mf % 4 == 3:
                    c = mf // 4
                    nc.sync.dma_start(out[b, :, 4 * c:4 * (c + 1), :],
                                      otf[:, 4 * c:4 * (c + 1), :])

    # Software pipeline: attend for pair p is emitted after the scores of
    # pair p+1, so the scores->softmax->transpose latency of p is hidden
    # under p+1's score matmuls instead of stalling the attend ldweights.
    pending = None
    for rep in range(reps):
        for half in range(NPAIR):
            if rep + half > 0:
                # PSUM-free PE activity across any wave-boundary wait:
                # standalone weight loads keep the clock-ramp monitor fed
                for _ in range(10):
                    nc.tensor.ldweights(ident_t[:])
            state = emit_scores(half)
            if pending is not None:
                emit_attend(*pending, last_wave=False)
            pending = (half,) + state
    emit_attend(*pending, last_wave=True)


def _get_program(reps=1):
    key = ("nc", reps)
    if key in _CACHE:
        return _CACHE[key]
    nc = bacc.Bacc("TRN2", target_bir_lowering=False, debug=False,
                   num_devices=NCORES)
    wa_d = nc.dram_tensor("walpha", [V, F], F16, kind="ExternalInput")
    vt_d = nc.dram_tensor("vt", [V, I], F16, kind="ExternalInput")
    vf_d = nc.dram_tensor("vf", [NPAIR, 128, FT, 2 * R], F16,
                          kind="ExternalInput")
    vft_d = nc.dram_tensor("vft", [BL, R, F], F16, kind="ExternalInput")
    id_d = nc.dram_tensor("ident", [128, 128], F16, kind="ExternalInput")
    out_d = nc.dram_tensor("out", [BL, 128, FT, I], F16,
                           kind="ExternalOutput")

    with tile.TileContext(nc) as tc, ExitStack() as ctx:
        _build_body(nc, tc, ctx, wa_d.ap(), vt_d.ap(), vf_d.ap(),
                    vft_d.ap(), id_d.ap(), out_d.ap(), reps)
    nc.compile()
    _CACHE[key] = nc
    return nc


def _prep_inputs(visual_features, v, W_alpha):
    vf = np.asarray(visual_features, dtype=np.float32)
    v = np.asarray(v, dtype=np.float32)
    W = np.asarray(W_alpha, dtype=np.float32)

    walpha16 = np.ascontiguousarray(W).astype(np.float16)          # [V, F]
    vt16 = np.ascontiguousarray(v.T).astype(np.float16)            # [V, I]
    # [b, f, r] -> [bp, p=128, t=16, j*196+r]: batch-paired, per-partition
    # contiguous DMA layout
    vf16 = np.ascontiguousarray(
        vf.reshape(B // 2, 2, FT, 128, R).transpose(0, 3, 2, 1, 4)
        .reshape(B // 2, 128, FT, 2 * R)).astype(np.float16)
    vft16 = np.ascontiguousarray(vf.transpose(0, 2, 1)).astype(np.float16)

    in_maps = []
    for c in range(NCORES):
        in_maps.append({
            "walpha": walpha16,
            "vt": vt16,
            "ident": np.eye(128, dtype=np.float16),
            "vf": np.ascontiguousarray(vf16[c * NPAIR:(c + 1) * NPAIR]),
            "vft": np.ascontiguousarray(vft16[c * BL:(c + 1) * BL]),
        })
    return in_maps


def kernel(visual_features, v, W_alpha):
    nc = _get_program()
    in_maps = _prep_inputs(visual_features, v, W_alpha)
    res = None
    for attempt in range(3):
        try:
            res = bass_utils.run_bass_kernel_spmd(
                nc, in_maps, core_ids=list(range(NCORES)))
            break
        except Exception:
            # transient NRT_EXEC_UNIT_UNRECOVERABLE wedges have been seen on
            # this fabric; a re-dispatch typically succeeds
            if attempt == 2:
                raise
    outs = [res.results[c]["out"] for c in range(NCORES)]
    buf = np.concatenate(outs, axis=0)          # [B, p=128, t=16, I]
    full = buf.transpose(0, 3, 2, 1).reshape(B, I, F)   # f = t*128 + p
    return np.ascontiguousarray(full).astype(np.float32)


# revision 61
# speedup vs baseline: 1.2815x; 1.0021x over previous
"""Trainium2 Bass kernel for attribute visual attention.

Computes, for each batch b:
    q      = v @ W_alpha                  # [i, f]
    scores = q @ vf[b]                    # [i, r]
    atten  = softmax(scores, axis=r)
    out[b] = atten @ vf[b].T              # [i, f]

Sharding: data-parallel over batch b across 8 NeuronCores (8 batches per
core); v / W_alpha replicated. All matmuls run in fp16 (full PE rate on
TRN2) with fp32 PSUM accumulation; softmax statistics in fp32.

Layout notes:
- The attend matmul contracts over r, which must live on SBUF partitions
  for both operands; the host passes visual_features twice — [f, r] for
  the scores matmul and pre-transposed [r, f] for the attend matmul. The
  small e = exp(scores - max) matrix is transposed on-chip on the PE.
- Batches are processed in PAIRS for the scores matmul (rhs = two
  batches side by side, N=392): halves the number of PE instructions and
  stationary-weight loads.
- Bulk HBM traffic uses SWDGE (gpsimd); weights/identity/outputs use
  HWDGE. DMA emission follows consumption order (weights, then per wave
  vf -> vft) with no blanket priority overrides, so the tile scheduler's
  priority heap preserves it; the PE is then fed continuously, which also
  keeps the DVFS p-state at full clock (an idle gap > ~4 us resets the
  ramp and the next matmul burst runs at the slow p-state).
- Streaming pools are allocated ahead of the phase-0 weight pool so their
  SBUF ranges are disjoint (address reuse would chain wave-0 loads behind
  the end of phase 0); a long Pool-engine memset (dhold) keeps the bulk
  SWDGE queue off the shared DMA engines while the weights stream in.
- Phase 1 is software-pipelined one wave deep: the attend matmuls of pair
  p are emitted after the scores of pair p+1, hiding the per-pair
  scores -> softmax -> transpose latency chain under score matmuls.
- Softmax normalization is applied while atten is i-partitioned; output
  store DMAs are issued per 4-f-tile chunk as soon as the chunk's
  PSUM->SBUF copies land, shortening the drain tail.
"""

import numpy as np
from contextlib import ExitStack

import concourse.bass as bass
import concourse.tile as tile
import concourse.bass_utils as bass_utils
from concourse import bacc, mybir

# Problem shapes (hardcoded per contest contract).
B, F, R, I, V = 64, 2048, 196, 312, 300
NCORES = 8
BL = B // NCORES          # 8 batches per core
NPAIR = BL // 2           # 4 batch-pairs per core
FT = F // 128             # 16 f-tiles
I_TILES = ((0, 128), (128, 128), (256, 56))
KV_TILES = ((0, 128), (128, 128), (256, 44))    # v=300
KR_TILES = ((0, 128), (128, 68))                # r=196
import os as _os
WARMUP = int(_os.environ.get("K_WARMUP", "45"))
                          # junk matmuls: finish the clock ramp AND bridge the
                          # weight-DMA window so phase 0 starts at full clock
DHOLD = int(_os.environ.get("K_DHOLD", "4200"))
VFP_BUFS = int(_os.environ.get("K_VFP", "3"))
VFTP_BUFS = int(_os.environ.get("K_VFTP", "3"))
OPSUM_BUFS = int(_os.environ.get("K_OPSUM", "4"))
QPSUM_BUFS = int(_os.environ.get("K_QPSUM", "4"))

F16 = mybir.dt.float16
F32 = mybir.dt.float32

_CACHE = {}


def _build_body(nc, tc, ctx, wa, vt, vf, vft, ident, out, reps):
    # Streaming input pools FIRST so their SBUF ranges are disjoint from the
    # phase-0 weight pool: otherwise wave-0 loads inherit an address-reuse
    # dependency on the end of phase 0 and the DMA pipeline stalls ~17 us.
    vfp = ctx.enter_context(tc.tile_pool(name="vf", bufs=VFP_BUFS))
    vftp = ctx.enter_context(tc.tile_pool(name="vft", bufs=VFTP_BUFS))
    qtp = ctx.enter_context(tc.tile_pool(name="qt", bufs=1))
    ident_t = qtp.tile([128, 128], F16, tag="ident", name="ident")

    # PE warm-up on a memset tile (no DMA dependency): junk matmuls from
    # ~t=0 so the clock ramp completes while the weight loads are in flight
    junk_t = qtp.tile([128, 128], F16, tag="junk", name="junk")
    nc.vector.memset(junk_t[:], 0.25)

    with tc.tile_pool(name="wupsum", bufs=1, space=bass.MemorySpace.PSUM) as wup:
        wu = wup.tile([128, 128], F32, tag="wu", name="wu")
        for w in range(WARMUP):
            nc.tensor.matmul(wu[:], junk_t[:], junk_t[:],
                             start=(w == 0), stop=(w == WARMUP - 1))
    # PSUM-free PE activity bridging the warmup -> phase-0 weight wait: on
    # real hardware the DVFS monitor needs sustained work to hold the clock
    for _ in range(30):
        nc.tensor.ldweights(junk_t[:])

    # Phase-1 PSUM pools allocated before qpsum so the scores accumulators
    # get banks disjoint from phase 0's; qpsum's banks are recycled by the
    # attend accumulators (opsum), whose first write comes well after the
    # last phase-0 read.
    spsum = ctx.enter_context(
        tc.tile_pool(name="spsum", bufs=2, space=bass.MemorySpace.PSUM))
    tpsum = ctx.enter_context(
        tc.tile_pool(name="tpsum", bufs=1, space=bass.MemorySpace.PSUM))

    # ---- Phase 0: qT[f, i] = (v @ W_alpha).T via lhsT=W_alpha, rhs=v.T ----
    # Weight DMAs in k-major order (vt, then per-k wa chunks) so the
    # mf-loop's k=0 matmuls unblock after ~half the weight bytes.
    qt_t = []
    with tc.tile_pool(name="const", bufs=1) as const, \
         tc.tile_pool(name="qpsum", bufs=QPSUM_BUFS, space=bass.MemorySpace.PSUM) as qpsum:
        vt_t, wa_t = [], []
        for k, (v0, vs) in enumerate(KV_TILES):
            t = const.tile([vs, I], F16, tag=f"vt{k}")
            nc.sync.dma_start(t[:], vt[v0:v0 + vs, :])
            vt_t.append(t)
        for k, (v0, vs) in enumerate(KV_TILES):
            wa_t.append(const.tile([vs, F], F16, tag=f"wa{k}", name=f"wa{k}"))
        for c in range(2):
            for k, (v0, vs) in enumerate(KV_TILES):
                nc.sync.dma_start(wa_t[k][:, c * 1024:(c + 1) * 1024],
                                  wa[v0:v0 + vs, c * 1024:(c + 1) * 1024])
        # identity (for the PE transposes, first needed at scores time)
        # loads after the weights so it doesn't crowd the HWDGE head
        nc.sync.dma_start(ident_t[:], ident[:])

        for mf in range(FT):
            qp = qpsum.tile([128, I], F32, tag="qp")
            for k, (v0, vs) in enumerate(KV_TILES):
                nc.tensor.matmul(qp[:], wa_t[k][:, mf * 128:(mf + 1) * 128],
                                 vt_t[k][:], start=(k == 0), stop=(k == 2))
            q = qtp.tile([128, I], F16, tag=f"qt{mf}")
            if mf % 2 == 0:
                nc.scalar.copy(q[:], qp[:])
            else:
                nc.vector.tensor_copy(q[:], qp[:])
            qt_t.append(q)

    # Hold the SWDGE bulk queue off the shared DMA engines while the
    # (phase-0-critical) weight DMAs stream in: one long Pool-engine memset
    # emitted ahead of the first prep. Both are ready at t=0, so the
    # scheduler's priority heap keeps the memset first; the weights then get
    # the DMA engines exclusively for the first ~5 us.
    dhold = qtp.tile([128, DHOLD], F16, tag="dhold", name="dhold")
    nc.gpsimd.memset(dhold[:], 0.0)

    # ---- Phase 1: per batch-pair attention ----
    esp = ctx.enter_context(tc.tile_pool(name="es", bufs=6))
    attp = ctx.enter_context(tc.tile_pool(name="atT", bufs=3))
    outp = ctx.enter_context(tc.tile_pool(name="out", bufs=3))
    stat = ctx.enter_context(tc.tile_pool(name="stat", bufs=8))
    opsum = ctx.enter_context(
        tc.tile_pool(name="opsum", bufs=OPSUM_BUFS, space=bass.MemorySpace.PSUM))

    def emit_scores(half):
        # vf pair tile [128, t, j*196+r], then vft per batch — emitted
        # in consumption order on the SWDGE queue
        vf_t = vfp.tile([128, FT, 2 * R], F16, tag="vf", name="vf0")
        for c in range(4):
            nc.gpsimd.dma_start(vf_t[:, 4 * c:4 * (c + 1), :],
                                vf[half, :, 4 * c:4 * (c + 1), :])
        vft_t = {}
        for j in range(2):
            b = 2 * half + j
            for kr, (r0, rs) in enumerate(KR_TILES):
                vv = vftp.tile([rs, F], F16, tag=f"vft{kr}{j}",
                               name=f"vft{kr}{j}")
                nc.gpsimd.dma_start(vv[:], vft[b, r0:r0 + rs, :])
                vft_t[(j, kr)] = vv

        esT_full = [
            [attp.tile([rs, I], F16, tag=f"esT{kr}{j}",
                       name=f"esT{kr}{j}")
             for kr, (r0, rs) in enumerate(KR_TILES)]
            for j in range(2)]
        for mi, (i0, isz) in enumerate(I_TILES):
            sp = spsum.tile([isz, 2, R], F32, tag="sp", name="sp")
            for kf in range(FT):
                nc.tensor.matmul(
                    sp[:], qt_t[kf][:, i0:i0 + isz],
                    vf_t[:, kf, :].rearrange("p (j r) -> p j r", j=2),
                    start=(kf == 0), stop=(kf == FT - 1))

            negmax = stat.tile([isz, 2], F32, tag="negmax")
            with tc.high_priority():
                nc.vector.tensor_reduce(negmax[:], sp[:],
                                        axis=mybir.AxisListType.X,
                                        op=mybir.AluOpType.max, negate=True)
            sums = stat.tile([isz, 2], F32, tag="sums")
            rcp = stat.tile([isz, 2], F32, tag="rcp")
            for j in range(2):
                es = esp.tile([128, R], F16, tag="es")
                att = esp.tile([128, R], F16, tag="att")
                with tc.high_priority():
                    nc.scalar.activation(es[:isz, 0:R], sp[:, j, :],
                                         mybir.ActivationFunctionType.Exp,
                                         bias=negmax[:, j:j + 1],
                                         scale=1.0,
                                         accum_out=sums[:, j:j + 1])
                    nc.vector.reciprocal(rcp[:, j:j + 1],
                                         sums[:, j:j + 1])
                    # normalize while atten is still i-partitioned
                    nc.vector.tensor_scalar_mul(att[:isz, :],
                                                es[:isz, :],
                                                rcp[:, j:j + 1])

                # transpose atten -> attenT[r, i-slice] on the PE
                # (transpose-mode matmul against identity); accumulate
                # the full [r, 312] attenT in SBUF across i-tiles
                for kr, (r0, rs) in enumerate(KR_TILES):
                    tp = tpsum.tile([rs, isz], F16, tag=f"tp{kr}",
                                    name=f"tp{kr}")
                    with tc.high_priority():
                        nc.tensor.transpose(tp[:], att[:isz, r0:r0 + rs],
                                            ident_t[0:isz, 0:isz])
                        nc.vector.tensor_copy(
                            esT_full[j][kr][:, i0:i0 + isz], tp[:])
        return vft_t, esT_full

    def emit_attend(half, vft_t, esT_full, last_wave):
        # attend (transposed output): outT[f, i] = vfT.T @ attenT,
        # M=f (16 exact tiles), N=i=312 -- no tile waste. Output DMA
        # per 4-f-tile chunk as soon as its copies land; the final batch
        # stores its last chunk as two 2-tile pieces (shorter drain tail).
        for j in range(2):
            b = 2 * half + j
            otf = outp.tile([128, FT, I], F16, tag=f"otf{j}",
                            name=f"otf{j}")
            for mf in range(FT):
                op_ = opsum.tile([128, I], F32, tag="op", name="op")
                for kr, (r0, rs) in enumerate(KR_TILES):
                    nc.tensor.matmul(
                        op_[:],
                        vft_t[(j, kr)][:, mf * 128:(mf + 1) * 128],
                        esT_full[j][kr][:],
                        start=(kr == 0), stop=(kr == 1))
                if mf % 2 == 0:
                    nc.scalar.copy(otf[:, mf, :], op_[:])
                else:
                    nc.vector.tensor_copy(otf[:, mf, :], op_[:])
                if last_wave and mf >= 12:
                    if mf % 2 == 1:
                        c = mf // 2
                        nc.sync.dma_start(out[b, :, 2 * c:2 * (c + 1), :],
                                          otf[:, 2 * c:2 * (c + 1), :])
                elif mf % 4 == 3:
                    c = mf // 4
                    nc.sync.dma_start(out[b, :, 4 * c:4 * (c + 1), :],
                                      otf[:, 4 * c:4 * (c + 1), :])

    # Software pipeline: attend for pair p is emitted after the scores of
    # pair p+1, so the scores->softmax->transpose latency of p is hidden
    # under p+1's score matmuls instead of stalling the attend ldweights.
    pending = None
    for rep in range(reps):
        for half in range(NPAIR):
            if rep + half > 0:
                # PSUM-free PE activity across any wave-boundary wait:
                # standalone weight loads keep the clock-ramp monitor fed
                for _ in range(10):
                    nc.tensor.ldweights(ident_t[:])
            state = emit_scores(half)
            if pending is not None:
                emit_attend(*pending, last_wave=False)
            pending = (half,) + state
    emit_attend(*pending, last_wave=True)


def _get_program(reps=1):
    key = ("nc", reps)
    if key in _CACHE:
        return _CACHE[key]
    nc = bacc.Bacc("TRN2", target_bir_lowering=False, debug=False,
                   num_devices=NCORES)
    wa_d = nc.dram_tensor("walpha", [V, F], F16, kind="ExternalInput")
    vt_d = nc.dram_tensor("vt", [V, I], F16, kind="ExternalInput")
    vf_d = nc.dram_tensor("vf", [NPAIR, 128, FT, 2 * R], F16,
                          kind="ExternalInput")
    vft_d = nc.dram_tensor("vft", [BL, R, F], F16, kind="ExternalInput")
    id_d = nc.dram_tensor("ident", [128, 128], F16, kind="ExternalInput")
    out_d = nc.dram_tensor("out", [BL, 128, FT, I], F16,
                           kind="ExternalOutput")

    with tile.TileContext(nc) as tc, ExitStack() as ctx:
        _build_body(nc, tc, ctx, wa_d.ap(), vt_d.ap(), vf_d.ap(),
                    vft_d.ap(), id_d.ap(), out_d.ap(), reps)
    nc.compile()
    _CACHE[key] = nc
    return nc


def _prep_inputs(visual_features, v, W_alpha):
    vf = np.asarray(visual_features, dtype=np.float32)
    v = np.asarray(v, dtype=np.float32)
    W = np.asarray(W_alpha, dtype=np.float32)

    walpha16 = np.ascontiguousarray(W).astype(np.float16)          # [V, F]
    vt16 = np.ascontiguousarray(v.T).astype(np.float16)            # [V, I]
    # [b, f, r] -> [bp, p=128, t=16, j*196+r]: batch-paired, per-partition
    # contiguous DMA layout
    vf16 = np.ascontiguousarray(
        vf.reshape(B // 2, 2, FT, 128, R).transpose(0, 3, 2, 1, 4)
        .reshape(B // 2, 128, FT, 2 * R)).astype(np.float16)
    vft16 = np.ascontiguousarray(vf.transpose(0, 2, 1)).astype(np.float16)

    in_maps = []
    for c in range(NCORES):
        in_maps.append({
            "walpha": walpha16,
            "vt": vt16,
            "ident": np.eye(128, dtype=np.float16),
            "vf": np.ascontiguousarray(vf16[c * NPAIR:(c + 1) * NPAIR]),
            "vft": np.ascontiguousarray(vft16[c * BL:(c + 1) * BL]),
        })
    return in_maps


def kernel(visual_features, v, W_alpha):
    nc = _get_program()
    in_maps = _prep_inputs(visual_features, v, W_alpha)
    res = None
    for attempt in range(3):
        try:
            res = bass_utils.run_bass_kernel_spmd(
                nc, in_maps, core_ids=list(range(NCORES)))
            break
        except Exception:
            # transient NRT_EXEC_UNIT_UNRECOVERABLE wedges have been seen on
            # this fabric; a re-dispatch typically succeeds
            if attempt == 2:
                raise
    outs = [res.results[c]["out"] for c in range(NCORES)]
    buf = np.concatenate(outs, axis=0)          # [B, p=128, t=16, I]
    full = buf.transpose(0, 3, 2, 1).reshape(B, I, F)   # f = t*128 + p
    return np.ascontiguousarray(full).astype(np.float32)


# revision 64
# speedup vs baseline: 1.2880x; 1.0051x over previous
"""Trainium2 Bass kernel for attribute visual attention.

Computes, for each batch b:
    q      = v @ W_alpha                  # [i, f]
    scores = q @ vf[b]                    # [i, r]
    atten  = softmax(scores, axis=r)
    out[b] = atten @ vf[b].T              # [i, f]

Sharding: data-parallel over batch b across 8 NeuronCores (8 batches per
core); v / W_alpha replicated. All matmuls run in fp16 (full PE rate on
TRN2) with fp32 PSUM accumulation; softmax statistics in fp32.

Layout notes:
- The attend matmul contracts over r, which must live on SBUF partitions
  for both operands; the host passes visual_features twice — [f, r] for
  the scores matmul and pre-transposed [r, f] for the attend matmul. The
  small e = exp(scores - max) matrix is transposed on-chip on the PE.
- Batches are processed in PAIRS for the scores matmul (rhs = two
  batches side by side, N=392): halves the number of PE instructions and
  stationary-weight loads.
- Bulk HBM traffic uses SWDGE (gpsimd); weights/identity/outputs use
  HWDGE. DMA emission follows consumption order (weights, then per wave
  vf -> vft) with no blanket priority overrides, so the tile scheduler's
  priority heap preserves it; the PE is then fed continuously, which also
  keeps the DVFS p-state at full clock (an idle gap > ~4 us resets the
  ramp and the next matmul burst runs at the slow p-state).
- Streaming pools are allocated ahead of the phase-0 weight pool so their
  SBUF ranges are disjoint (address reuse would chain wave-0 loads behind
  the end of phase 0); a long Pool-engine memset (dhold) keeps the bulk
  SWDGE queue off the shared DMA engines while the weights stream in.
- Phase 1 is software-pipelined one wave deep: the attend matmuls of pair
  p are emitted after the scores of pair p+1, hiding the per-pair
  scores -> softmax -> transpose latency chain under score matmuls.
- Softmax normalization is applied while atten is i-partitioned; output
  store DMAs are issued per 4-f-tile chunk as soon as the chunk's
  PSUM->SBUF copies land, shortening the drain tail.
"""

import numpy as np
from contextlib import ExitStack

import concourse.bass as bass
import concourse.tile as tile
import concourse.bass_utils as bass_utils
from concourse import bacc, mybir

# Problem shapes (hardcoded per contest contract).
B, F, R, I, V = 64, 2048, 196, 312, 300
NCORES = 8
BL = B // NCORES          # 8 batches per core
NPAIR = BL // 2           # 4 batch-pairs per core
FT = F // 128             # 16 f-tiles
I_TILES = ((0, 128), (128, 128), (256, 56))
KV_TILES = ((0, 128), (128, 128), (256, 44))    # v=300
KR_TILES = ((0, 128), (128, 68))                # r=196
import os as _os
WARMUP = int(_os.environ.get("K_WARMUP", "45"))
                          # junk matmuls: finish the clock ramp AND bridge the
                          # weight-DMA window so phase 0 starts at full clock
DHOLD = int(_os.environ.get("K_DHOLD", "4200"))
VFP_BUFS = int(_os.environ.get("K_VFP", "3"))
VFTP_BUFS = int(_os.environ.get("K_VFTP", "3"))
OPSUM_BUFS = int(_os.environ.get("K_OPSUM", "4"))
QPSUM_BUFS = int(_os.environ.get("K_QPSUM", "4"))

F16 = mybir.dt.float16
F32 = mybir.dt.float32

_CACHE = {}


def _build_body(nc, tc, ctx, wvt, vf, vft, ident, out, reps):
    # Streaming input pools FIRST so their SBUF ranges are disjoint from the
    # phase-0 weight pool: otherwise wave-0 loads inherit an address-reuse
    # dependency on the end of phase 0 and the DMA pipeline stalls ~17 us.
    vfp = ctx.enter_context(tc.tile_pool(name="vf", bufs=VFP_BUFS))
    vftp = ctx.enter_context(tc.tile_pool(name="vft", bufs=VFTP_BUFS))
    qtp = ctx.enter_context(tc.tile_pool(name="qt", bufs=1))
    ident_t = qtp.tile([128, 128], F16, tag="ident", name="ident")

    # PE warm-up on a memset tile (no DMA dependency): junk matmuls from
    # ~t=0 so the clock ramp completes while the weight loads are in flight
    junk_t = qtp.tile([128, 128], F16, tag="junk", name="junk")
    nc.vector.memset(junk_t[:], 0.25)

    with tc.tile_pool(name="wupsum", bufs=1, space=bass.MemorySpace.PSUM) as wup:
        wu = wup.tile([128, 128], F32, tag="wu", name="wu")
        for w in range(WARMUP):
            nc.tensor.matmul(wu[:], junk_t[:], junk_t[:],
                             start=(w == 0), stop=(w == WARMUP - 1))
    # PSUM-free PE activity bridging the warmup -> phase-0 weight wait: on
    # real hardware the DVFS monitor needs sustained work to hold the clock
    for _ in range(30):
        nc.tensor.ldweights(junk_t[:])

    # Phase-1 PSUM pools allocated before qpsum so the scores accumulators
    # get banks disjoint from phase 0's; qpsum's banks are recycled by the
    # attend accumulators (opsum), whose first write comes well after the
    # last phase-0 read.
    spsum = ctx.enter_context(
        tc.tile_pool(name="spsum", bufs=2, space=bass.MemorySpace.PSUM))
    tpsum = ctx.enter_context(
        tc.tile_pool(name="tpsum", bufs=1, space=bass.MemorySpace.PSUM))

    # ---- Phase 0: qT[f, i] = (v @ W_alpha).T via lhsT=W_alpha, rhs=v.T ----
    # Weight DMAs in k-major order (vt, then per-k wa chunks) so the
    # mf-loop's k=0 matmuls unblock after ~half the weight bytes.
    qt_t = []
    with tc.tile_pool(name="const", bufs=1) as const, \
         tc.tile_pool(name="qpsum", bufs=QPSUM_BUFS, space=bass.MemorySpace.PSUM) as qpsum:
        # vt and W_alpha arrive host-packed per k-slice ([vs, 312+2048]):
        # 6 weight DMAs instead of 9 -- the HWDGE generator (625 ns apiece,
        # serial) paces this stream, so fewer items land the weights sooner
        HALF = (I + F) // 2
        wvt_t = []
        for k, (v0, vs) in enumerate(KV_TILES):
            wvt_t.append(const.tile([vs, I + F], F16, tag=f"wvt{k}",
                                    name=f"wvt{k}"))
        for c in range(2):
            for k, (v0, vs) in enumerate(KV_TILES):
                nc.sync.dma_start(wvt_t[k][:, c * HALF:(c + 1) * HALF],
                                  wvt[v0:v0 + vs, c * HALF:(c + 1) * HALF])
        vt_t = [t[:, 0:I] for t in wvt_t]
        wa_t = [t[:, I:I + F] for t in wvt_t]
        # identity (for the PE transposes, first needed at scores time)
        # loads after the weights so it doesn't crowd the HWDGE head
        nc.sync.dma_start(ident_t[:], ident[:])

        for mf in range(FT):
            qp = qpsum.tile([128, I], F32, tag="qp")
            for k, (v0, vs) in enumerate(KV_TILES):
                nc.tensor.matmul(qp[:], wa_t[k][:, mf * 128:(mf + 1) * 128],
                                 vt_t[k][:], start=(k == 0), stop=(k == 2))
            q = qtp.tile([128, I], F16, tag=f"qt{mf}")
            if mf % 2 == 0:
                nc.scalar.copy(q[:], qp[:])
            else:
                nc.vector.tensor_copy(q[:], qp[:])
            qt_t.append(q)

    # Hold the SWDGE bulk queue off the shared DMA engines while the
    # (phase-0-critical) weight DMAs stream in: one long Pool-engine memset
    # emitted ahead of the first prep. Both are ready at t=0, so the
    # scheduler's priority heap keeps the memset first; the weights then get
    # the DMA engines exclusively for the first ~5 us.
    dhold = qtp.tile([128, DHOLD], F16, tag="dhold", name="dhold")
    nc.gpsimd.memset(dhold[:], 0.0)

    # ---- Phase 1: per batch-pair attention ----
    esp = ctx.enter_context(tc.tile_pool(name="es", bufs=6))
    attp = ctx.enter_context(tc.tile_pool(name="atT", bufs=3))
    outp = ctx.enter_context(tc.tile_pool(name="out", bufs=3))
    stat = ctx.enter_context(tc.tile_pool(name="stat", bufs=8))
    opsum = ctx.enter_context(
        tc.tile_pool(name="opsum", bufs=OPSUM_BUFS, space=bass.MemorySpace.PSUM))

    def emit_scores(half):
        # vf pair tile [128, t, j*196+r], then vft per batch — emitted
        # in consumption order on the SWDGE queue
        vf_t = vfp.tile([128, FT, 2 * R], F16, tag="vf", name="vf0")
        for c in range(4):
            nc.gpsimd.dma_start(vf_t[:, 4 * c:4 * (c + 1), :],
                                vf[half, :, 4 * c:4 * (c + 1), :])
        vft_t = {}
        for j in range(2):
            b = 2 * half + j
            for kr, (r0, rs) in enumerate(KR_TILES):
                vv = vftp.tile([rs, F], F16, tag=f"vft{kr}{j}",
                               name=f"vft{kr}{j}")
                nc.gpsimd.dma_start(vv[:], vft[b, r0:r0 + rs, :])
                vft_t[(j, kr)] = vv

        esT_full = [
            [attp.tile([rs, I], F16, tag=f"esT{kr}{j}",
                       name=f"esT{kr}{j}")
             for kr, (r0, rs) in enumerate(KR_TILES)]
            for j in range(2)]
        for mi, (i0, isz) in enumerate(I_TILES):
            sp = spsum.tile([isz, 2, R], F32, tag="sp", name="sp")
            for kf in range(FT):
                nc.tensor.matmul(
                    sp[:], qt_t[kf][:, i0:i0 + isz],
                    vf_t[:, kf, :].rearrange("p (j r) -> p j r", j=2),
                    start=(kf == 0), stop=(kf == FT - 1))

            negmax = stat.tile([isz, 2], F32, tag="negmax")
            with tc.high_priority():
                nc.vector.tensor_reduce(negmax[:], sp[:],
                                        axis=mybir.AxisListType.X,
                                        op=mybir.AluOpType.max, negate=True)
            sums = stat.tile([isz, 2], F32, tag="sums")
            rcp = stat.tile([isz, 2], F32, tag="rcp")
            for j in range(2):
                es = esp.tile([128, R], F16, tag="es")
                att = esp.tile([128, R], F16, tag="att")
                with tc.high_priority():
                    nc.scalar.activation(es[:isz, 0:R], sp[:, j, :],
                                         mybir.ActivationFunctionType.Exp,
                                         bias=negmax[:, j:j + 1],
                                         scale=1.0,
                                         accum_out=sums[:, j:j + 1])
                    nc.vector.reciprocal(rcp[:, j:j + 1],
                                         sums[:, j:j + 1])
                    # normalize while atten is still i-partitioned
                    nc.vector.tensor_scalar_mul(att[:isz, :],
                                                es[:isz, :],
                                                rcp[:, j:j + 1])

                # transpose atten -> attenT[r, i-slice] on the PE
                # (transpose-mode matmul against identity); accumulate
                # the full [r, 312] attenT in SBUF across i-tiles
                for kr, (r0, rs) in enumerate(KR_TILES):
                    tp = tpsum.tile([rs, isz], F16, tag=f"tp{kr}",
                                    name=f"tp{kr}")
                    with tc.high_priority():
                        nc.tensor.transpose(tp[:], att[:isz, r0:r0 + rs],
                                            ident_t[0:isz, 0:isz])
                        nc.vector.tensor_copy(
                            esT_full[j][kr][:, i0:i0 + isz], tp[:])
        return vft_t, esT_full

    def emit_attend(half, vft_t, esT_full, last_wave):
        # attend (transposed output): outT[f, i] = vfT.T @ attenT,
        # M=f (16 exact tiles), N=i=312 -- no tile waste. Output DMA
        # per 4-f-tile chunk as soon as its copies land; the final batch
        # stores its last chunk as two 2-tile pieces (shorter drain tail).
        for j in range(2):
            b = 2 * half + j
            otf = outp.tile([128, FT, I], F16, tag=f"otf{j}",
                            name=f"otf{j}")
            for mf in range(FT):
                op_ = opsum.tile([128, I], F32, tag="op", name="op")
                for kr, (r0, rs) in enumerate(KR_TILES):
                    nc.tensor.matmul(
                        op_[:],
                        vft_t[(j, kr)][:, mf * 128:(mf + 1) * 128],
                        esT_full[j][kr][:],
                        start=(kr == 0), stop=(kr == 1))
                if mf % 2 == 0:
                    nc.scalar.copy(otf[:, mf, :], op_[:])
                else:
                    nc.vector.tensor_copy(otf[:, mf, :], op_[:])
                if last_wave and mf >= 12:
                    if mf % 2 == 1:
                        c = mf // 2
                        nc.sync.dma_start(out[b, :, 2 * c:2 * (c + 1), :],
                                          otf[:, 2 * c:2 * (c + 1), :])
                elif mf % 4 == 3:
                    c = mf // 4
                    nc.sync.dma_start(out[b, :, 4 * c:4 * (c + 1), :],
                                      otf[:, 4 * c:4 * (c + 1), :])

    # Software pipeline: attend for pair p is emitted after the scores of
    # pair p+1, so the scores->softmax->transpose latency of p is hidden
    # under p+1's score matmuls instead of stalling the attend ldweights.
    pending = None
    for rep in range(reps):
        for half in range(NPAIR):
            if rep + half > 0:
                # PSUM-free PE activity across any wave-boundary wait:
                # standalone weight loads keep the clock-ramp monitor fed
                for _ in range(10):
                    nc.tensor.ldweights(ident_t[:])
            state = emit_scores(half)
            if pending is not None:
                emit_attend(*pending, last_wave=False)
            pending = (half,) + state
    emit_attend(*pending, last_wave=True)


def _get_program(reps=1):
    key = ("nc", reps)
    if key in _CACHE:
        return _CACHE[key]
    nc = bacc.Bacc("TRN2", target_bir_lowering=False, debug=False,
                   num_devices=NCORES)
    wvt_d = nc.dram_tensor("wvt", [V, I + F], F16, kind="ExternalInput")
    vf_d = nc.dram_tensor("vf", [NPAIR, 128, FT, 2 * R], F16,
                          kind="ExternalInput")
    vft_d = nc.dram_tensor("vft", [BL, R, F], F16, kind="ExternalInput")
    id_d = nc.dram_tensor("ident", [128, 128], F16, kind="ExternalInput")
    out_d = nc.dram_tensor("out", [BL, 128, FT, I], F16,
                           kind="ExternalOutput")

    with tile.TileContext(nc) as tc, ExitStack() as ctx:
        _build_body(nc, tc, ctx, wvt_d.ap(), vf_d.ap(),
                    vft_d.ap(), id_d.ap(), out_d.ap(), reps)
    nc.compile()
    _CACHE[key] = nc
    return nc


def _prep_inputs(visual_features, v, W_alpha):
    vf = np.asarray(visual_features, dtype=np.float32)
    v = np.asarray(v, dtype=np.float32)
    W = np.asarray(W_alpha, dtype=np.float32)

    # [V, 312+2048]: v.T columns then W_alpha columns, packed per partition
    wvt16 = np.ascontiguousarray(
        np.concatenate([v.T, W], axis=1)).astype(np.float16)
    # [b, f, r] -> [bp, p=128, t=16, j*196+r]: batch-paired, per-partition
    # contiguous DMA layout
    vf16 = np.ascontiguousarray(
        vf.reshape(B // 2, 2, FT, 128, R).transpose(0, 3, 2, 1, 4)
        .reshape(B // 2, 128, FT, 2 * R)).astype(np.float16)
    vft16 = np.ascontiguousarray(vf.transpose(0, 2, 1)).astype(np.float16)

    in_maps = []
    for c in range(NCORES):
        in_maps.append({
            "wvt": wvt16,
            "ident": np.eye(128, dtype=np.float16),
            "vf": np.ascontiguousarray(vf16[c * NPAIR:(c + 1) * NPAIR]),
            "vft": np.ascontiguousarray(vft16[c * BL:(c + 1) * BL]),
        })
    return in_maps


def kernel(visual_features, v, W_alpha):
    nc = _get_program()
    in_maps = _prep_inputs(visual_features, v, W_alpha)
    res = None
    for attempt in range(3):
        try:
            res = bass_utils.run_bass_kernel_spmd(
                nc, in_maps, core_ids=list(range(NCORES)))
            break
        except Exception:
            # transient NRT_EXEC_UNIT_UNRECOVERABLE wedges have been seen on
            # this fabric; a re-dispatch typically succeeds
            if attempt == 2:
                raise
    outs = [res.results[c]["out"] for c in range(NCORES)]
    buf = np.concatenate(outs, axis=0)          # [B, p=128, t=16, I]
    full = buf.transpose(0, 3, 2, 1).reshape(B, I, F)   # f = t*128 + p
    return np.ascontiguousarray(full).astype(np.float32)


# revision 76
# speedup vs baseline: 1.3031x; 1.0117x over previous
"""Trainium2 Bass kernel for attribute visual attention.

Computes, for each batch b:
    q      = v @ W_alpha                  # [i, f]
    scores = q @ vf[b]                    # [i, r]
    atten  = softmax(scores, axis=r)
    out[b] = atten @ vf[b].T              # [i, f]

Sharding: data-parallel over batch b across 8 NeuronCores (8 batches per
core); v / W_alpha replicated. All matmuls run in fp16 (full PE rate on
TRN2) with fp32 PSUM accumulation; softmax statistics in fp32.

Layout notes:
- The attend matmul contracts over r, which must live on SBUF partitions
  for both operands; the host passes visual_features twice — [f, r] for
  the scores matmul and pre-transposed [r, f] for the attend matmul. The
  small e = exp(scores - max) matrix is transposed on-chip on the PE.
- Batches are processed in PAIRS for the scores matmul (rhs = two
  batches side by side, N=392): halves the number of PE instructions and
  stationary-weight loads.
- Bulk HBM traffic uses SWDGE (gpsimd); weights/identity/outputs use
  HWDGE. DMA emission follows consumption order (weights, then per wave
  vf -> vft) with no blanket priority overrides, so the tile scheduler's
  priority heap preserves it; the PE is then fed continuously, which also
  keeps the DVFS p-state at full clock (an idle gap > ~4 us resets the
  ramp and the next matmul burst runs at the slow p-state).
- Streaming pools are allocated ahead of the phase-0 weight pool so their
  SBUF ranges are disjoint (address reuse would chain wave-0 loads behind
  the end of phase 0); a long Pool-engine memset (dhold) keeps the bulk
  SWDGE queue off the shared DMA engines while the weights stream in.
- Phase 1 is software-pipelined one wave deep: the attend matmuls of pair
  p are emitted after the scores of pair p+1, hiding the per-pair
  scores -> softmax -> transpose latency chain under score matmuls.
- Softmax normalization is applied while atten is i-partitioned; output
  store DMAs are issued per 4-f-tile chunk as soon as the chunk's
  PSUM->SBUF copies land, shortening the drain tail.
"""

import numpy as np
from contextlib import ExitStack

import concourse.bass as bass
import concourse.tile as tile
import concourse.bass_utils as bass_utils
from concourse import bacc, mybir

# Problem shapes (hardcoded per contest contract).
B, F, R, I, V = 64, 2048, 196, 312, 300
NCORES = 8
BL = B // NCORES          # 8 batches per core
NPAIR = BL // 2           # 4 batch-pairs per core
FT = F // 128             # 16 f-tiles
I_TILES = ((0, 128), (128, 128), (256, 56))
KV_TILES = ((0, 128), (128, 128), (256, 44))    # v=300
KR_TILES = ((0, 128), (128, 68))                # r=196
import os as _os
WARMUP = int(_os.environ.get("K_WARMUP", "42"))
                          # junk matmuls: finish the clock ramp AND bridge the
                          # weight-DMA window so phase 0 starts at full clock
DHOLD = int(_os.environ.get("K_DHOLD", "3000"))
VFP_BUFS = int(_os.environ.get("K_VFP", "3"))
VFTP_BUFS = int(_os.environ.get("K_VFTP", "3"))
OPSUM_BUFS = int(_os.environ.get("K_OPSUM", "4"))
QPSUM_BUFS = int(_os.environ.get("K_QPSUM", "4"))

F16 = mybir.dt.float16
F32 = mybir.dt.float32

_CACHE = {}


def _build_body(nc, tc, ctx, wvt, vf, vft, ident, out, reps):
    # Streaming input pools FIRST so their SBUF ranges are disjoint from the
    # phase-0 weight pool: otherwise wave-0 loads inherit an address-reuse
    # dependency on the end of phase 0 and the DMA pipeline stalls ~17 us.
    vfp = ctx.enter_context(tc.tile_pool(name="vf", bufs=VFP_BUFS))
    vftp = ctx.enter_context(tc.tile_pool(name="vft", bufs=VFTP_BUFS))
    qtp = ctx.enter_context(tc.tile_pool(name="qt", bufs=1))
    ident_t = qtp.tile([128, 128], F16, tag="ident", name="ident")

    # PE warm-up on a memset tile (no DMA dependency): junk matmuls from
    # ~t=0 so the clock ramp completes while the weight loads are in flight
    junk_t = qtp.tile([128, 128], F16, tag="junk", name="junk")
    nc.vector.memset(junk_t[:], 0.25)

    with tc.tile_pool(name="wupsum", bufs=1, space=bass.MemorySpace.PSUM) as wup:
        wu = wup.tile([128, 128], F32, tag="wu", name="wu")
        for w in range(WARMUP):
            nc.tensor.matmul(wu[:], junk_t[:], junk_t[:],
                             start=(w == 0), stop=(w == WARMUP - 1))
    # PSUM-free PE activity bridging the warmup -> phase-0 weight wait: on
    # real hardware the DVFS monitor needs sustained work to hold the clock
    for _ in range(30):
        nc.tensor.ldweights(junk_t[:])

    # Phase-1 PSUM pools allocated before qpsum so the scores accumulators
    # get banks disjoint from phase 0's; qpsum's banks are recycled by the
    # attend accumulators (opsum), whose first write comes well after the
    # last phase-0 read.
    spsum = ctx.enter_context(
        tc.tile_pool(name="spsum", bufs=2, space=bass.MemorySpace.PSUM))
    tpsum = ctx.enter_context(
        tc.tile_pool(name="tpsum", bufs=1, space=bass.MemorySpace.PSUM))

    # ---- Phase 0: qT[f, i] = (v @ W_alpha).T via lhsT=W_alpha, rhs=v.T ----
    # Weight DMAs in k-major order (vt, then per-k wa chunks) so the
    # mf-loop's k=0 matmuls unblock after ~half the weight bytes.
    qt_t = []
    with tc.tile_pool(name="const", bufs=1) as const, \
         tc.tile_pool(name="qpsum", bufs=QPSUM_BUFS, space=bass.MemorySpace.PSUM) as qpsum:
        # vt and W_alpha arrive host-packed per k-slice ([vs, 312+2048]):
        # 6 weight DMAs instead of 9 -- the HWDGE generator (625 ns apiece,
        # serial) paces this stream, so fewer items land the weights sooner
        SPLITS = (0, I + 640, I + F)
        wvt_t = []
        for k, (v0, vs) in enumerate(KV_TILES):
            wvt_t.append(const.tile([vs, I + F], F16, tag=f"wvt{k}",
                                    name=f"wvt{k}"))
        for c in range(2):
            lo, hi = SPLITS[c], SPLITS[c + 1]
            for k, (v0, vs) in enumerate(KV_TILES):
                nc.sync.dma_start(wvt_t[k][:, lo:hi],
                                  wvt[v0:v0 + vs, lo:hi])
        vt_t = [t[:, 0:I] for t in wvt_t]
        wa_t = [t[:, I:I + F] for t in wvt_t]
        # identity (for the PE transposes, first needed at scores time)
        # loads after the weights so it doesn't crowd the HWDGE head
        nc.sync.dma_start(ident_t[:], ident[:])

        for mf in range(FT):
            qp = qpsum.tile([128, I], F32, tag="qp")
            for k, (v0, vs) in enumerate(KV_TILES):
                nc.tensor.matmul(qp[:], wa_t[k][:, mf * 128:(mf + 1) * 128],
                                 vt_t[k][:], start=(k == 0), stop=(k == 2))
            q = qtp.tile([128, I], F16, tag=f"qt{mf}")
            if mf % 2 == 0:
                nc.scalar.copy(q[:], qp[:])
            else:
                nc.vector.tensor_copy(q[:], qp[:])
            qt_t.append(q)

    # Hold the SWDGE bulk queue off the shared DMA engines while the
    # (phase-0-critical) weight DMAs stream in: one long Pool-engine memset
    # emitted ahead of the first prep. Both are ready at t=0, so the
    # scheduler's priority heap keeps the memset first; the weights then get
    # the DMA engines exclusively for the first ~5 us.
    dhold = qtp.tile([128, DHOLD], F16, tag="dhold", name="dhold")
    nc.gpsimd.memset(dhold[:], 0.0)

    # ---- Phase 1: per batch-pair attention ----
    esp = ctx.enter_context(tc.tile_pool(name="es", bufs=6))
    attp = ctx.enter_context(tc.tile_pool(name="atT", bufs=3))
    outp = ctx.enter_context(tc.tile_pool(name="out", bufs=3))
    stat = ctx.enter_context(tc.tile_pool(name="stat", bufs=8))
    opsum = ctx.enter_context(
        tc.tile_pool(name="opsum", bufs=OPSUM_BUFS, space=bass.MemorySpace.PSUM))

    def emit_scores(half):
        # vf pair tile [128, t, j*196+r], then vft per batch — emitted
        # in consumption order on the SWDGE queue
        vf_t = vfp.tile([128, FT, 2 * R], F16, tag="vf", name="vf0")
        for c in range(4):
            nc.gpsimd.dma_start(vf_t[:, 4 * c:4 * (c + 1), :],
                                vf[half, :, 4 * c:4 * (c + 1), :])
        vft_t = {}
        for j in range(2):
            b = 2 * half + j
            for kr, (r0, rs) in enumerate(KR_TILES):
                vv = vftp.tile([rs, F], F16, tag=f"vft{kr}{j}",
                               name=f"vft{kr}{j}")
                nc.gpsimd.dma_start(vv[:], vft[b, r0:r0 + rs, :])
                vft_t[(j, kr)] = vv

        esT_full = [
            [attp.tile([rs, I], F16, tag=f"esT{kr}{j}",
                       name=f"esT{kr}{j}")
             for kr, (r0, rs) in enumerate(KR_TILES)]
            for j in range(2)]
        for mi, (i0, isz) in enumerate(I_TILES):
            sp = spsum.tile([isz, 2, R], F32, tag="sp", name="sp")
            for kf in range(FT):
                nc.tensor.matmul(
                    sp[:], qt_t[kf][:, i0:i0 + isz],
                    vf_t[:, kf, :].rearrange("p (j r) -> p j r", j=2),
                    start=(kf == 0), stop=(kf == FT - 1))

            negmax = stat.tile([isz, 2], F32, tag="negmax")
            with tc.high_priority():
                nc.vector.tensor_reduce(negmax[:], sp[:],
                                        axis=mybir.AxisListType.X,
                                        op=mybir.AluOpType.max, negate=True)
            sums = stat.tile([isz, 2], F32, tag="sums")
            rcp = stat.tile([isz, 2], F32, tag="rcp")
            for j in range(2):
                es = esp.tile([128, R], F16, tag="es")
                att = esp.tile([128, R], F16, tag="att")
                with tc.high_priority():
                    nc.scalar.activation(es[:isz, 0:R], sp[:, j, :],
                                         mybir.ActivationFunctionType.Exp,
                                         bias=negmax[:, j:j + 1],
                                         scale=1.0,
                                         accum_out=sums[:, j:j + 1])
                    nc.vector.reciprocal(rcp[:, j:j + 1],
                                         sums[:, j:j + 1])
                    # normalize while atten is still i-partitioned
                    nc.vector.tensor_scalar_mul(att[:isz, :],
                                                es[:isz, :],
                                                rcp[:, j:j + 1])

                # transpose atten -> attenT[r, i-slice] on the PE
                # (transpose-mode matmul against identity); accumulate
                # the full [r, 312] attenT in SBUF across i-tiles
                for kr, (r0, rs) in enumerate(KR_TILES):
                    tp = tpsum.tile([rs, isz], F16, tag=f"tp{kr}",
                                    name=f"tp{kr}")
                    with tc.high_priority():
                        nc.tensor.transpose(tp[:], att[:isz, r0:r0 + rs],
                                            ident_t[0:isz, 0:isz])
                        nc.vector.tensor_copy(
                            esT_full[j][kr][:, i0:i0 + isz], tp[:])
        return vft_t, esT_full

    def emit_attend(half, vft_t, esT_full, last_wave):
        # attend (transposed output): outT[f, i] = vfT.T @ attenT,
        # M=f (16 exact tiles), N=i=312 -- no tile waste. Output DMA
        # per 4-f-tile chunk as soon as its copies land; the final batch
        # stores its last chunk as two 2-tile pieces (shorter drain tail).
        for j in range(2):
            b = 2 * half + j
            otf = outp.tile([128, FT, I], F16, tag=f"otf{j}",
                            name=f"otf{j}")
            for mf in range(FT):
                op_ = opsum.tile([128, I], F32, tag="op", name="op")
                for kr, (r0, rs) in enumerate(KR_TILES):
                    nc.tensor.matmul(
                        op_[:],
                        vft_t[(j, kr)][:, mf * 128:(mf + 1) * 128],
                        esT_full[j][kr][:],
                        start=(kr == 0), stop=(kr == 1))
                if mf % 2 == 0:
                    nc.scalar.copy(otf[:, mf, :], op_[:])
                else:
                    nc.vector.tensor_copy(otf[:, mf, :], op_[:])
                if last_wave and mf >= 12:
                    if mf % 2 == 1:
                        c = mf // 2
                        nc.sync.dma_start(out[b, :, 2 * c:2 * (c + 1), :],
                                          otf[:, 2 * c:2 * (c + 1), :])
                elif mf % 4 == 3:
                    c = mf // 4
                    nc.sync.dma_start(out[b, :, 4 * c:4 * (c + 1), :],
                                      otf[:, 4 * c:4 * (c + 1), :])

    # Software pipeline: attend for pair p is emitted after the scores of
    # pair p+1, so the scores->softmax->transpose latency of p is hidden
    # under p+1's score matmuls instead of stalling the attend ldweights.
    pending = None
    for rep in range(reps):
        for half in range(NPAIR):
            if rep + half > 0:
                # PSUM-free PE activity across any wave-boundary wait:
                # standalone weight loads keep the clock-ramp monitor fed
                for _ in range(10):
                    nc.tensor.ldweights(ident_t[:])
            state = emit_scores(half)
            if pending is not None:
                emit_attend(*pending, last_wave=False)
            pending = (half,) + state
    emit_attend(*pending, last_wave=True)


def _get_program(reps=1):
    key = ("nc", reps)
    if key in _CACHE:
        return _CACHE[key]
    nc = bacc.Bacc("TRN2", target_bir_lowering=False, debug=False,
                   num_devices=NCORES)
    wvt_d = nc.dram_tensor("wvt", [V, I + F], F16, kind="ExternalInput")
    vf_d = nc.dram_tensor("vf", [NPAIR, 128, FT, 2 * R], F16,
                          kind="ExternalInput")
    vft_d = nc.dram_tensor("vft", [BL, R, F], F16, kind="ExternalInput")
    id_d = nc.dram_tensor("ident", [128, 128], F16, kind="ExternalInput")
    out_d = nc.dram_tensor("out", [BL, 128, FT, I], F16,
                           kind="ExternalOutput")

    with tile.TileContext(nc) as tc, ExitStack() as ctx:
        _build_body(nc, tc, ctx, wvt_d.ap(), vf_d.ap(),
                    vft_d.ap(), id_d.ap(), out_d.ap(), reps)
    nc.compile()
    _CACHE[key] = nc
    return nc


def _prep_inputs(visual_features, v, W_alpha):
    vf = np.asarray(visual_features, dtype=np.float32)
    v = np.asarray(v, dtype=np.float32)
    W = np.asarray(W_alpha, dtype=np.float32)

    # [V, 312+2048]: v.T columns then W_alpha columns, packed per partition
    wvt16 = np.ascontiguousarray(
        np.concatenate([v.T, W], axis=1)).astype(np.float16)
    # [b, f, r] -> [bp, p=128, t=16, j*196+r]: batch-paired, per-partition
    # contiguous DMA layout
    vf16 = np.ascontiguousarray(
        vf.reshape(B // 2, 2, FT, 128, R).transpose(0, 3, 2, 1, 4)
        .reshape(B // 2, 128, FT, 2 * R)).astype(np.float16)
    vft16 = np.ascontiguousarray(vf.transpose(0, 2, 1)).astype(np.float16)

    in_maps = []
    for c in range(NCORES):
        in_maps.append({
            "wvt": wvt16,
            "ident": np.eye(128, dtype=np.float16),
            "vf": np.ascontiguousarray(vf16[c * NPAIR:(c + 1) * NPAIR]),
            "vft": np.ascontiguousarray(vft16[c * BL:(c + 1) * BL]),
        })
    return in_maps


def kernel(visual_features, v, W_alpha):
    nc = _get_program()
    in_maps = _prep_inputs(visual_features, v, W_alpha)
    res = None
    for attempt in range(3):
        try:
            res = bass_utils.run_bass_kernel_spmd(
                nc, in_maps, core_ids=list(range(NCORES)))
            break
        except Exception:
            # transient NRT_EXEC_UNIT_UNRECOVERABLE wedges have been seen on
            # this fabric; a re-dispatch typically succeeds
            if attempt == 2:
                raise
    outs = [res.results[c]["out"] for c in range(NCORES)]
    buf = np.concatenate(outs, axis=0)          # [B, p=128, t=16, I]
    full = buf.transpose(0, 3, 2, 1).reshape(B, I, F)   # f = t*128 + p
    return np.ascontiguousarray(full).astype(np.float32)
